# revision 1
# baseline (speedup 1.0000x reference)
"""Trainium2 Bass kernel for nn_DistanceLayer (gaussian-prior distance attention).

Math: out[b,i] = sum_j softmax_j(q_i.k_j * MD^-0.5 * prior(j-i))[j] * (j-i)

Key observation: the gaussian prior (std=1) underflows so fast in f32 that
for |j-i| outside a small band the f32 score is exactly 0, so exp(score)
is exactly 1.0.  The softmax row then consists of a small band of
"interesting" values plus a uniform far field whose sums are known in
closed form.  We therefore compute only a narrow window of scores around
the diagonal on the PE and fold the far field in with exact host-side
constants:

    T0_i = (N - win) + sum_window exp(s)            (denominator)
    T1_i = C1_i + sum_window exp(s)*c + ws_i * sum_window exp(s)
    out_i = T1_i / T0_i - i

where C1_i = sum_all_j j - sum_window_i j (exact integers < 2^24, exact in
f32) and ws_i is the window start of row i's 64-row half-tile.  In-window
far entries have score exactly 0 (prior premultiplied in, 0 outside the
band) and contribute exp(0)=1, which the constants account for.

Layout: rows are processed as 64-row halves packed two-per-partition-dim
(windows stay narrow: win = 64 + band + pad), and two 128-row tiles are
batched per postprocessing pass ([P, 2*win] multiply/exp, 3D reduces for
the per-tile sums) to amortize fixed per-op engine costs.

Sharding: pure data-parallel over batch B=8 across the 8 cores; each core
holds the full (small) QK weights and computes its own [N] output row.
"""

import sys

sys.path.insert(0, "/opt/trn_rl_repo")

import ml_dtypes
import numpy as np

import concourse.bacc as bacc
import concourse.tile as tile
from concourse import mybir
from concourse.bass_utils import run_bass_kernel_spmd

B, N, D, MD = 8, 2048, 256, 128
NCORES = 8
P = 128
HR = P // 2  # 64-row half-tiles
NT = N // P  # 16 row tiles
NPAIR = NT // 2  # 8 postprocessing pairs
DCH = D // P  # 2 contraction chunks for the projections
PROJ_CHUNK = 512
NPC = N // PROJ_CHUNK  # 4 projection column chunks
PI = 3.1415926  # matches reference
F32 = mybir.dt.float32
BF16 = mybir.dt.bfloat16

_cache = {}
# exposed for test harness profiling: (nc, in_maps)
last_run = None


def _plan_band(prior_mean, prior_std):
    """f32 prior over every offset, exactly as the reference computes it,
    and the band of offsets whose scores can round exp() away from 1.0."""
    d = np.arange(-(N - 1), N, dtype=np.float32)
    ps = np.float32(prior_std)
    pm = np.float32(prior_mean)
    prior = (
        np.float32(1.0)
        / ps
        / np.sqrt(np.float32(2.0) * np.float32(PI))
        * np.exp(np.float32(-0.5) * (d - pm) ** 2 / ps**2)
    ).astype(np.float32)
    # |score| <= |prior| * |q.k*scale| ; bound the latter by 1024 (actual
    # max is ~7 for these glorot inputs).  exp(x) rounds to 1.0f for
    # |x| < 2^-26; use 2^-27 for margin.
    sig = np.abs(prior) * 1024.0 >= 2.0**-27
    if not sig.any():
        dlo, dhi = 0, 0
    else:
        dlo = int(d[sig].min())
        dhi = int(d[sig].max())
    return prior, dlo, dhi


def _window_geometry(dlo, dhi):
    """Per-64-row-half window starts ws2[32] plus deduplicated per-pair
    prior patterns.  Pattern key for pair g (tiles 2g, 2g+1) is the tuple
    of its four half-window offsets relative to the pair's base row."""
    span = dhi - dlo
    win = HR + span + 1
    win = max(80, ((win + 15) // 16) * 16)
    assert win <= 512, f"prior band too wide for banded kernel: {dlo}..{dhi}"
    extra = win - (HR + span)
    ws2 = []
    for h in range(2 * NT):
        ws = min(max(h * HR + dlo - extra // 2, 0), N - win)
        lo_need = max(0, h * HR + dlo)
        hi_need = min(N - 1, h * HR + HR - 1 + dhi)
        assert ws <= lo_need and hi_need < ws + win, (h, ws, lo_need, hi_need)
        ws2.append(ws)
    pair_keys = []
    for g in range(NPAIR):
        base = 2 * P * g
        pair_keys.append(tuple(ws2[4 * g + i] - base for i in range(4)))
    key_vals = sorted(set(pair_keys))
    key_idx = [key_vals.index(k) for k in pair_keys]
    return win, ws2, key_vals, key_idx


def _build(win, ws2, key_idx, n_pat):
    nc = bacc.Bacc()

    # f32 consts: bq | bk | c1 | wsm | ii | j0pair ; bf16: pair prior patterns
    CW = 2 + 3 * NT + 2 * win
    O_BQ, O_BK = 0, 1
    O_C1 = 2
    O_WS = O_C1 + NT
    O_II = O_WS + NT
    O_J0 = O_II + NT
    CW16 = n_pat * 2 * win

    w2_d = nc.dram_tensor("w2", [P, 2 * DCH * MD], BF16, kind="ExternalInput")
    xt_d = nc.dram_tensor("xt", [NPC, P, DCH * PROJ_CHUNK], BF16, kind="ExternalInput")
    cs_d = nc.dram_tensor("cst", [P, CW], F32, kind="ExternalInput")
    c16_d = nc.dram_tensor("cst16", [P, CW16], BF16, kind="ExternalInput")
    y_d = nc.dram_tensor("y", [P, NT], F32, kind="ExternalOutput")

    with tile.TileContext(nc) as tc:
        with (
            tc.tile_pool(name="const", bufs=1) as const,
            tc.tile_pool(name="psum_proj", bufs=3, space="PSUM") as psum_proj,
            tc.tile_pool(name="psum_band", bufs=3, space="PSUM") as psum_band,
            tc.tile_pool(name="band_sp", bufs=2) as sp_pool,
            tc.tile_pool(name="band_e", bufs=2) as e_pool,
            tc.tile_pool(name="band_ej", bufs=2) as ej_pool,
            tc.tile_pool(name="comb", bufs=1) as comb,
        ):
            # ---- engine warmups (run while DMAs are in flight) ----
            # PE: junk matmuls keep the PE busy until the input DMAs land,
            # flipping the HAM clock gate to 8/8 before the real matmuls.
            # ACT: one tiny Exp pulls the 1.3us ACT_TABLE_LOAD off the
            # critical path.
            wtile = const.tile([P, PROJ_CHUNK], BF16, tag="warm_w")
            nc.vector.memset(wtile, 0.0)
            for _ in range(7):
                wps = psum_proj.tile([P, PROJ_CHUNK], F32, tag="proj")
                nc.tensor.matmul(
                    wps,
                    lhsT=wtile[:, :P],
                    rhs=wtile[:, :PROJ_CHUNK],
                    start=True,
                    stop=True,
                )
            wact_in = const.tile([P, 1], F32, tag="warm_a")
            nc.vector.memset(wact_in, 0.0)
            wact_out = const.tile([P, 1], F32, tag="warm_ao")
            nc.scalar.activation(
                out=wact_out, in_=wact_in, func=mybir.ActivationFunctionType.Exp
            )

            # ---- input DMAs; first ones go on the scalar queue so their
            # descriptor generation runs parallel to sync's preamble ----
            w2_s = const.tile([P, 2 * DCH * MD], BF16, tag="w2")
            nc.scalar.dma_start(out=w2_s, in_=w2_d[:, :])
            xts = []
            for i in range(NPC):
                t = const.tile([P, DCH * PROJ_CHUNK], BF16, tag=f"xt{i}")
                xts.append(t)
            nc.scalar.dma_start(out=xts[0], in_=xt_d[0])
            cs_s = const.tile([P, CW], F32, tag="cst")
            nc.scalar.dma_start(out=cs_s, in_=cs_d[:, :])
            c16_s = const.tile([P, CW16], BF16, tag="cst16")
            nc.scalar.dma_start(out=c16_s, in_=c16_d[:, :])
            for i in range(1, NPC):
                nc.sync.dma_start(out=xts[i], in_=xt_d[i])

            qT = const.tile([P, N], BF16, tag="qT")
            kT = const.tile([P, N], BF16, tag="kT")
            sum_e = const.tile([P, NT], F32, tag="sum_e")
            sum_ec = const.tile([P, NT], F32, tag="sum_ec")

            # ---- band pair: tiles 2g, 2g+1 share one [P, 2*win] pass ----
            def emit_pair(g):
                ps_s = psum_band.tile([P, 2 * win], F32, tag="band")
                for tb in range(2):  # tile within pair
                    t = 2 * g + tb
                    for hb in range(2):  # 64-row half on partitions
                        ws = ws2[2 * t + hb]
                        nc.tensor.matmul(
                            ps_s[hb * HR : (hb + 1) * HR, tb * win : (tb + 1) * win],
                            lhsT=qT[:, t * P + hb * HR : t * P + (hb + 1) * HR],
                            rhs=kT[:, ws : ws + win],
                            start=True,
                            stop=True,
                        )
                oi = key_idx[g]
                sp_t = sp_pool.tile([P, 2 * win], F32, tag="sp")
                nc.vector.tensor_mul(
                    sp_t, ps_s, c16_s[:, oi * 2 * win : (oi + 1) * 2 * win]
                )
                e_t = e_pool.tile([P, 2 * win], F32, tag="e")
                nc.scalar.activation(
                    out=e_t, in_=sp_t, func=mybir.ActivationFunctionType.Exp
                )
                ej_t = ej_pool.tile([P, 2 * win], F32, tag="ej")
                mul_eng = nc.vector if g == NPAIR - 1 else nc.gpsimd
                mul_eng.tensor_mul(ej_t, e_t, cs_s[:, O_J0 : O_J0 + 2 * win])
                nc.vector.tensor_reduce(
                    out=sum_e[:, 2 * g : 2 * g + 2],
                    in_=e_t[:].rearrange("p (t w) -> p t w", w=win),
                    axis=mybir.AxisListType.X,
                    op=mybir.AluOpType.add,
                )
                nc.vector.tensor_reduce(
                    out=sum_ec[:, 2 * g : 2 * g + 2],
                    in_=ej_t[:].rearrange("p (t w) -> p t w", w=win),
                    axis=mybir.AxisListType.X,
                    op=mybir.AluOpType.add,
                )

            # pair g needs both projections evicted through this chunk:
            def pair_chunk(g):
                hi = max(min(ws2[h] + win, N) for h in range(4 * g, 4 * g + 4))
                return max((2 * g + 1) // (PROJ_CHUNK // P), (hi - 1) // PROJ_CHUNK)

            pairs_after = {n4: [] for n4 in range(NPC)}
            for g in range(NPAIR):
                pairs_after[pair_chunk(g)].append(g)

            # ---- projections; chunk0 evictions split across ACT+DVE for
            # the fastest band unlock, later chunks all on ACT (the band
            # postprocessing now loads DVE+GpSimd more than ACT) ----
            def emit_chunk(n4, split_evict=False):
                for pj in range(2):  # 0=q, 1=k
                    b_s = cs_s[:, O_BQ + pj : O_BQ + pj + 1]
                    dstT = (qT, kT)[pj]
                    ps_t = psum_proj.tile([P, PROJ_CHUNK], F32, tag="proj")
                    for c in range(DCH):
                        nc.tensor.matmul(
                            ps_t,
                            lhsT=w2_s[:, (2 * pj + c) * MD : (2 * pj + c + 1) * MD],
                            rhs=xts[n4][:, c * PROJ_CHUNK : (c + 1) * PROJ_CHUNK],
                            start=(c == 0),
                            stop=(c == DCH - 1),
                        )
                    lo = n4 * PROJ_CHUNK
                    if split_evict:
                        half = PROJ_CHUNK // 2
                        nc.vector.tensor_scalar_add(
                            dstT[:, lo : lo + half], ps_t[:, :half], b_s
                        )
                        nc.scalar.activation(
                            out=dstT[:, lo + half : lo + PROJ_CHUNK],
                            in_=ps_t[:, half:],
                            func=mybir.ActivationFunctionType.Identity,
                            bias=b_s,
                            scale=1.0,
                        )
                    else:
                        nc.scalar.activation(
                            out=dstT[:, lo : lo + PROJ_CHUNK],
                            in_=ps_t,
                            func=mybir.ActivationFunctionType.Identity,
                            bias=b_s,
                            scale=1.0,
                        )

            # shift-by-one: pair MMs are emitted after the NEXT chunk's
            # matmuls so their evictions are already done (engine queues
            # are FIFO; a waiting matmul would stall the whole PE queue).
            emit_chunk(0, split_evict=True)
            emit_chunk(1)
            band_plan = []
            for n4 in range(2, NPC + 2):
                for g in pairs_after[n4 - 2]:
                    band_plan.append(("pair", g))
                if n4 < NPC:
                    band_plan.append(("chunk", n4))


            # ---- combine: out = (c1 + sum_ec + ws*sum_e)/(N-win+sum_e) - i ----
            c1_s = cs_s[:, O_C1 : O_C1 + NT]
            ws_s = cs_s[:, O_WS : O_WS + NT]
            ii_s = cs_s[:, O_II : O_II + NT]
            outv2 = comb.tile([P, NT], F32, tag="outv2")

            def emit_combine(sl):
                w = sl.stop - sl.start
                t0 = comb.tile([P, w], F32, tag="t0")
                nc.vector.tensor_scalar_add(t0, sum_e[:, sl], float(N - win))
                rec = comb.tile([P, w], F32, tag="rec")
                nc.vector.reciprocal(rec, t0)
                tmp = comb.tile([P, w], F32, tag="tmp")
                nc.vector.tensor_mul(tmp, ws_s[:, sl], sum_e[:, sl])
                num = comb.tile([P, w], F32, tag="num")
                nc.vector.tensor_add(num, c1_s[:, sl], sum_ec[:, sl])
                num2 = comb.tile([P, w], F32, tag="num2")
                nc.vector.tensor_add(num2, num, tmp)
                outv = comb.tile([P, w], F32, tag="outv")
                nc.vector.tensor_mul(outv, num2, rec)
                nc.vector.tensor_sub(outv2[:, sl], outv, ii_s[:, sl])

            # first-half combine hides under the last pairs
            for kind, v in band_plan:
                if kind == "pair":
                    emit_pair(v)
                    if v == NPAIR - 2:
                        emit_combine(slice(0, 8))
                else:
                    emit_chunk(v)
            emit_combine(slice(8, NT))
            nc.sync.dma_start(out=y_d[:, :], in_=outv2)

    nc.finalize()
    return nc


def kernel(x, Wq, bq, Wk, bk, prior_mean, prior_std):
    global last_run
    x = np.asarray(x, dtype=np.float32)
    Wq = np.asarray(Wq, dtype=np.float32)
    Wk = np.asarray(Wk, dtype=np.float32)
    bq = np.asarray(bq, dtype=np.float32)
    bk = np.asarray(bk, dtype=np.float32)

    prior, dlo, dhi = _plan_band(
        float(np.asarray(prior_mean)[0]), float(np.asarray(prior_std)[0])
    )
    win, ws2, key_vals, key_idx = _window_geometry(dlo, dhi)
    n_pat = len(key_vals)

    key = (win, tuple(ws2), tuple(key_idx))
    if key not in _cache:
        _cache[key] = _build(win, ws2, key_idx, n_pat)
    nc = _cache[key]

    bf = ml_dtypes.bfloat16
    scale = np.float32(MD**-0.5)

    # prior*scale pair patterns: [P, 2*win] per distinct 4-offset key.
    # value[p, tb*win + c] = prior[c + rel_ws[tb, hb] - 128*tb - p] * scale
    # where hb selects by partition half (p >= 64).
    p_idx = np.arange(P)[:, None]
    c_idx = np.arange(win)[None, :]
    pmat = np.zeros((P, n_pat * 2 * win), np.float32)
    for ki, rel in enumerate(key_vals):
        for tb in range(2):
            relcol = np.where(np.arange(P) < HR, rel[2 * tb], rel[2 * tb + 1])[:, None]
            dm = c_idx + relcol - 128 * tb - p_idx
            pmat[:, ki * 2 * win + tb * win : ki * 2 * win + (tb + 1) * win] = np.where(
                (dm >= dlo) & (dm <= dhi), prior[dm + N - 1] * scale, np.float32(0.0)
            ).astype(np.float32)

    sumj_all = float(N * (N - 1) // 2)
    c1 = np.zeros((P, NT), np.float32)
    wsm = np.zeros((P, NT), np.float32)
    ii = np.zeros((P, NT), np.float32)
    half_sel = np.arange(P) >= HR
    for t in range(NT):
        wsa, wsb = ws2[2 * t], ws2[2 * t + 1]
        wsv = np.where(half_sel, float(wsb), float(wsa))
        c1[:, t] = sumj_all - (win * wsv + win * (win - 1) // 2)
        wsm[:, t] = wsv
        ii[:, t] = t * P + np.arange(P)

    # consts: f32 = bq | bk | c1 | wsm | ii | j0pair ; bf16 = pair patterns
    j0pair = np.broadcast_to(
        np.tile(np.arange(win, dtype=np.float32), 2), (P, 2 * win)
    )
    cst = np.ascontiguousarray(
        np.concatenate(
            [bq.reshape(P, 1), bk.reshape(P, 1), c1, wsm, ii, j0pair], axis=1
        ).astype(np.float32)
    )
    cst16 = np.ascontiguousarray(pmat.astype(bf))

    # weights: wq chunks then wk chunks, [P, 4*MD]
    wq_h = Wq.reshape(DCH, P, MD).transpose(1, 0, 2).reshape(P, DCH * MD)
    wk_h = Wk.reshape(DCH, P, MD).transpose(1, 0, 2).reshape(P, DCH * MD)
    w2_h = np.ascontiguousarray(np.concatenate([wq_h, wk_h], axis=1)).astype(bf)

    in_maps = []
    for core in range(NCORES):
        xb = x[core]  # [N, D]
        # xt[n4, p, c*512 + j] = x[n4*512 + j, c*128 + p]
        xt_h = np.ascontiguousarray(
            xb.T.reshape(DCH, P, NPC, PROJ_CHUNK)
            .transpose(2, 1, 0, 3)
            .reshape(NPC, P, DCH * PROJ_CHUNK)
        ).astype(bf)
        in_maps.append({"xt": xt_h, "w2": w2_h, "cst": cst, "cst16": cst16})

    res = run_bass_kernel_spmd(nc, in_maps, list(range(NCORES)))
    last_run = (nc, in_maps)
    # y[p, t] = out[128t + p]  ->  out = y.T.flatten()
    out = np.stack(
        [res.results[c]["y"].T.reshape(-1) for c in range(NCORES)], axis=0
    )
    return out.astype(np.float32)



# revision 8
# speedup vs baseline: 1.0518x; 1.0518x over previous
"""Trainium2 Bass kernel for nn_DistanceLayer (gaussian-prior distance attention).

Math: out[b,i] = sum_j softmax_j(q_i.k_j * MD^-0.5 * prior(j-i))[j] * (j-i)

The gaussian prior (std=1) underflows so fast in f32 that outside a small
band |j-i| <= 7 the f32 score is exactly 0, so exp(score) is exactly 1.0.
Each softmax row is a small band of "interesting" values plus a uniform
far field with closed-form sums:

    T0_i = (N - win) + sum_window exp(s)            (denominator)
    T1_i = C1_i + sum_window exp(s)*jrel + ws_i * sum_window exp(s)
    out_i = T1_i / T0_i - i

where C1_i = sum_all_j j - sum_window_i j (exact ints in f32) and ws_i is
the window start of row i's 64-row half.  In-window far entries have
score exactly 0 (prior pattern is 0 outside the band) and contribute
exp(0)=1, which the constants account for.

v2 layout/schedule (vs the 31.6us v1):
  - x and the QK weights ship as fp8e4m3 (weights pre-scaled x8, the
    x64 undone in the prior pattern), halving input DMA bytes; input
    DMAs spread over 3 HW queues (gpsimd/vector/sync) so the first
    projection unlocks ~3us earlier.
  - bands processed in 4-tile groups ([P, 4*win] per pass) to amortize
    fixed per-op costs; everything after exp is bf16 for 2x DVE rates.
  - postprocessing spread across DVE/ACT/Pool so no engine exceeds ~5us:
    ACT does exp + 4 projection evictions, DVE does sp-mul/ej/reduces +
    2 evictions + reciprocal, Pool does memsets/2 sp-muls/2 evictions +
    most of the combine.
  - PE warmup junk matmuls ramp the clock (0.65->2.4GHz takes 3us of
    continuous busy) while DMAs land.

Sharding: pure data-parallel over batch B=8 across the 8 cores.
"""

import sys

sys.path.insert(0, "/opt/trn_rl_repo")

import ml_dtypes
import numpy as np

import concourse.bacc as bacc
import concourse.tile as tile
from concourse import mybir
from concourse.bass_utils import run_bass_kernel_spmd

B, N, D, MD = 8, 2048, 256, 128
NCORES = 8
P = 128
HR = P // 2  # 64-row halves
NT = N // P  # 16 row tiles
NG = 4  # band groups of 4 tiles
GT = NT // NG  # tiles per group
DCH = D // P  # 2 contraction chunks
PROJ_CHUNK = 512
NPC = N // PROJ_CHUNK  # 4 projection column chunks
PI = 3.1415926  # matches reference
WSCALE = 8.0  # fp8 weight pre-scale; pattern divides the x64 back out
F32 = mybir.dt.float32
BF16 = mybir.dt.bfloat16
F8 = mybir.dt.float8e4

_cache = {}
# exposed for test harness profiling: (nc, in_maps)
last_run = None


def _plan_band(prior_mean, prior_std):
    """f32 prior over every offset, exactly as the reference computes it,
    and the band of offsets whose scores can round exp() away from 1.0."""
    d = np.arange(-(N - 1), N, dtype=np.float32)
    ps = np.float32(prior_std)
    pm = np.float32(prior_mean)
    prior = (
        np.float32(1.0)
        / ps
        / np.sqrt(np.float32(2.0) * np.float32(PI))
        * np.exp(np.float32(-0.5) * (d - pm) ** 2 / ps**2)
    ).astype(np.float32)
    # |score| <= |prior| * |q.k*scale| ; bound the latter by 1024 (actual
    # max is ~7 for these glorot inputs).  exp(x) rounds to 1.0f for
    # |x| < 2^-26; use 2^-27 for margin.
    sig = np.abs(prior) * 1024.0 >= 2.0**-27
    if not sig.any():
        dlo, dhi = 0, 0
    else:
        dlo = int(d[sig].min())
        dhi = int(d[sig].max())
    return prior, dlo, dhi


def _window_geometry(dlo, dhi):
    """Per-64-row-half window starts ws2[32] plus deduplicated per-group
    prior patterns.  Pattern key for group g (tiles 4g..4g+3) is the tuple
    of its eight half-window offsets relative to the group's base row."""
    span = dhi - dlo
    win = HR + span + 1
    win = max(80, ((win + 15) // 16) * 16)
    assert win <= 448, f"prior band too wide for banded kernel: {dlo}..{dhi}"
    extra = win - (HR + span)
    ws2 = []
    for h in range(2 * NT):
        ws = min(max(h * HR + dlo - extra // 2, 0), N - win)
        lo_need = max(0, h * HR + dlo)
        hi_need = min(N - 1, h * HR + HR - 1 + dhi)
        assert ws <= lo_need and hi_need < ws + win, (h, ws, lo_need, hi_need)
        ws2.append(ws)
    gkeys = []
    for g in range(NG):
        base = GT * P * g
        gkeys.append(tuple(ws2[2 * GT * g + i] - base for i in range(2 * GT)))
    key_vals = sorted(set(gkeys))
    key_idx = [key_vals.index(k) for k in gkeys]
    return win, ws2, key_vals, key_idx


def _build(win, ws2, key_idx, n_pat, use_bias):
    nc = bacc.Bacc()
    GW = GT * win  # group width

    # f32 consts: c1 | wsm | ii | bq8 | bk8
    O_C1, O_WS, O_II, O_BQ = 0, NT, 2 * NT, 3 * NT
    CW = 3 * NT + 2
    # bf16 consts: n_pat group patterns then j0
    PJW = n_pat * GW + GW
    O_J0 = n_pat * GW

    xt_d = nc.dram_tensor("xt", [P, DCH * N], F8, kind="ExternalInput")
    w2_d = nc.dram_tensor("w2", [P, 2 * DCH * MD], F8, kind="ExternalInput")
    cs_d = nc.dram_tensor("cst", [P, CW], F32, kind="ExternalInput")
    pj_d = nc.dram_tensor("pj", [P, PJW], BF16, kind="ExternalInput")
    y_d = nc.dram_tensor("y", [P, NT], F32, kind="ExternalOutput")

    with tile.TileContext(nc) as tc:
        with (
            tc.tile_pool(name="const", bufs=1) as const,
            tc.tile_pool(name="psum_proj", bufs=3, space="PSUM") as psum_proj,
            tc.tile_pool(name="psum_band", bufs=3, space="PSUM") as psum_band,
            tc.tile_pool(name="band_sp", bufs=2) as sp_pool,
            tc.tile_pool(name="band_e", bufs=2) as e_pool,
            tc.tile_pool(name="band_ej", bufs=2) as ej_pool,
            tc.tile_pool(name="comb", bufs=1) as comb,
        ):
            # ---- engine warmups ----
            # PE: junk matmuls flip the HAM clock gate to 8/8 while the
            # input DMAs are in flight.  ACT: one tiny Exp pulls the 1.3us
            # ACT_TABLE_LOAD off the critical path.
            wtile = const.tile([P, PROJ_CHUNK], BF16, tag="warm_w")
            nc.gpsimd.memset(wtile, 0.0)
            wact_in = const.tile([P, 1], F32, tag="warm_a")
            nc.gpsimd.memset(wact_in, 0.0)
            wact_out = const.tile([P, 1], F32, tag="warm_ao")
            nc.scalar.activation(
                out=wact_out, in_=wact_in, func=mybir.ActivationFunctionType.Exp
            )

            # ---- input DMAs across 3 queues ----
            w2_s = const.tile([P, 2 * DCH * MD], F8, tag="w2")
            nc.sync.dma_start(out=w2_s, in_=w2_d[:, :])
            xt_s = const.tile([P, DCH * N], F8, tag="xt")
            half = N // 2
            # c0/c1 first halves unlock proj chunks 0-1; second halves 2-3.
            # only gpsimd/SP/ACT can issue DMAs; gpsimd's issue cost is lowest
            nc.gpsimd.dma_start(out=xt_s[:, 0:half], in_=xt_d[:, 0:half])
            nc.gpsimd.dma_start(out=xt_s[:, N : N + half], in_=xt_d[:, N : N + half])
            nc.sync.dma_start(out=xt_s[:, half:N], in_=xt_d[:, half:N])
            nc.scalar.dma_start(
                out=xt_s[:, N + half : 2 * N], in_=xt_d[:, N + half : 2 * N]
            )
            pj_s = const.tile([P, PJW], BF16, tag="pj")
            nc.gpsimd.dma_start(out=pj_s, in_=pj_d[:, :])
            cs_s = const.tile([P, CW], F32, tag="cst")
            nc.gpsimd.dma_start(out=cs_s, in_=cs_d[:, :])

            for _ in range(5):
                wps = psum_proj.tile([P, PROJ_CHUNK], F32, tag="proj")
                nc.tensor.matmul(
                    wps,
                    lhsT=wtile[:, :P],
                    rhs=wtile[:, :PROJ_CHUNK],
                    start=True,
                    stop=True,
                )

            qT = const.tile([P, N], BF16, tag="qT")
            kT = const.tile([P, N], BF16, tag="kT")
            sum_e = const.tile([P, NT], BF16, tag="sum_e")
            sum_ec = const.tile([P, NT], BF16, tag="sum_ec")

            # ---- projections: chunk n4, pj 0=q 1=k; evictions rotate
            # over ACT/DVE/Pool so no engine stalls the PE ----
            # gpsimd cannot read PSUM, so evictions and sp-muls are split
            # over ACT/DVE only; Pool gets the SBUF-only postprocessing
            EV = [  # (q0,k0,q1,k1,q2,k2,q3,k3) eviction engines
                "act", "dve", "act", "dve", "act", "dve", "act", "act",
            ]

            def emit_proj(n4, pj):
                dstT = (qT, kT)[pj]
                ps_t = psum_proj.tile([P, PROJ_CHUNK], F32, tag="proj")
                for c in range(DCH):
                    nc.tensor.matmul(
                        ps_t,
                        lhsT=w2_s[:, (2 * pj + c) * MD : (2 * pj + c + 1) * MD],
                        rhs=xt_s[:, c * N + n4 * PROJ_CHUNK : c * N + (n4 + 1) * PROJ_CHUNK],
                        start=(c == 0),
                        stop=(c == DCH - 1),
                    )
                dst = dstT[:, n4 * PROJ_CHUNK : (n4 + 1) * PROJ_CHUNK]
                eng = EV[2 * n4 + pj]
                if use_bias:
                    b_s = cs_s[:, O_BQ + pj : O_BQ + pj + 1]
                    if eng == "act":
                        nc.scalar.activation(
                            out=dst, in_=ps_t,
                            func=mybir.ActivationFunctionType.Identity,
                            bias=b_s, scale=1.0,
                        )
                    elif eng == "dve":
                        nc.vector.tensor_scalar_add(dst, ps_t, b_s)
                else:
                    if eng == "act":
                        nc.scalar.copy(out=dst, in_=ps_t)
                    else:
                        nc.vector.tensor_copy(dst, ps_t)

            # ---- band group: tiles 4g..4g+3 in one [P, 4*win] pass ----
            def emit_group(g):
                ps_s = psum_band.tile([P, GW], F32, tag="band")
                for tb in range(GT):
                    t = GT * g + tb
                    for hb in range(2):
                        ws = ws2[2 * t + hb]
                        nc.tensor.matmul(
                            ps_s[hb * HR : (hb + 1) * HR, tb * win : (tb + 1) * win],
                            lhsT=qT[:, t * P + hb * HR : t * P + (hb + 1) * HR],
                            rhs=kT[:, ws : ws + win],
                            start=True,
                            stop=True,
                        )
                oi = key_idx[g]
                pat = pj_s[:, oi * GW : (oi + 1) * GW]
                sp_t = sp_pool.tile([P, GW], BF16, tag="sp")
                nc.vector.tensor_mul(sp_t, ps_s, pat)
                e_t = e_pool.tile([P, GW], BF16, tag="e")
                nc.scalar.activation(
                    out=e_t, in_=sp_t, func=mybir.ActivationFunctionType.Exp
                )
                ej_t = ej_pool.tile([P, GW], BF16, tag="ej")
                nc.gpsimd.tensor_mul(ej_t, e_t, pj_s[:, O_J0 : O_J0 + GW])
                # bf16 sums: worst-case 0.4% of ~8e3 is ~0.02 abs on a
                # +-1023-scale output (tolerance 2e-2 rel) — safe, and the
                # 2-byte dtype doubles the DVE reduce rate.
                with nc.allow_low_precision("bf16 window sums, ~1e-5 rel out err"):
                    nc.vector.tensor_reduce(
                        out=sum_e[:, GT * g : GT * (g + 1)],
                        in_=e_t[:].rearrange("p (t w) -> p t w", w=win),
                        axis=mybir.AxisListType.X,
                        op=mybir.AluOpType.add,
                    )
                    nc.vector.tensor_reduce(
                        out=sum_ec[:, GT * g : GT * (g + 1)],
                        in_=ej_t[:].rearrange("p (t w) -> p t w", w=win),
                        axis=mybir.AxisListType.X,
                        op=mybir.AluOpType.add,
                    )

            # ---- combine: out = (c1 + sum_ec + ws*sum_e)/(N-win+sum_e) - i ----
            c1_s = cs_s[:, O_C1 : O_C1 + NT]
            ws_s = cs_s[:, O_WS : O_WS + NT]
            ii_s = cs_s[:, O_II : O_II + NT]
            outv2 = comb.tile([P, NT], F32, tag="outv2")

            def emit_combine(sl):
                w = sl.stop - sl.start
                t0 = comb.tile([P, w], F32, tag="t0")
                nc.gpsimd.tensor_scalar_add(t0, sum_e[:, sl], float(N - win))
                rec = comb.tile([P, w], F32, tag="rec")
                nc.vector.reciprocal(rec, t0)
                tmp = comb.tile([P, w], F32, tag="tmp")
                nc.gpsimd.tensor_mul(tmp, ws_s[:, sl], sum_e[:, sl])
                num = comb.tile([P, w], F32, tag="num")
                nc.gpsimd.tensor_add(num, c1_s[:, sl], sum_ec[:, sl])
                num2 = comb.tile([P, w], F32, tag="num2")
                nc.gpsimd.tensor_add(num2, num, tmp)
                outv = comb.tile([P, w], F32, tag="outv")
                nc.vector.tensor_mul(outv, num2, rec)
                nc.gpsimd.tensor_sub(outv2[:, sl], outv, ii_s[:, sl])
                nc.sync.dma_start(out=y_d[:, sl], in_=outv2[:, sl])

            emit_proj(0, 0)
            emit_proj(0, 1)
            emit_proj(1, 0)
            emit_proj(1, 1)
            emit_group(0)
            emit_proj(2, 0)
            emit_proj(2, 1)
            emit_group(1)
            emit_proj(3, 0)
            emit_proj(3, 1)
            emit_group(2)
            emit_combine(slice(0, 8))
            emit_group(3)
            emit_combine(slice(8, NT))

    nc.finalize()
    return nc


def kernel(x, Wq, bq, Wk, bk, prior_mean, prior_std):
    global last_run
    x = np.asarray(x, dtype=np.float32)
    Wq = np.asarray(Wq, dtype=np.float32)
    Wk = np.asarray(Wk, dtype=np.float32)
    bq = np.asarray(bq, dtype=np.float32)
    bk = np.asarray(bk, dtype=np.float32)

    prior, dlo, dhi = _plan_band(
        float(np.asarray(prior_mean)[0]), float(np.asarray(prior_std)[0])
    )
    win, ws2, key_vals, key_idx = _window_geometry(dlo, dhi)
    n_pat = len(key_vals)
    use_bias = bool(np.any(bq != 0.0) or np.any(bk != 0.0))
    GW = GT * win

    key = (win, tuple(ws2), tuple(key_idx), use_bias)
    if key not in _cache:
        _cache[key] = _build(win, ws2, key_idx, n_pat, use_bias)
    nc = _cache[key]

    bf = ml_dtypes.bfloat16
    f8 = ml_dtypes.float8_e4m3
    scale = np.float32(MD**-0.5) / np.float32(WSCALE * WSCALE)

    # prior*scale group patterns: [P, GW] per distinct 8-offset key.
    # value[p, tb*win + c] = prior[c + rel[2tb + (p>=64)] - 128*tb - p] * scale
    p_idx = np.arange(P)[:, None]
    c_idx = np.arange(win)[None, :]
    half_sel = np.arange(P) >= HR
    pj = np.zeros((P, n_pat * GW + GW), np.float32)
    for ki, rel in enumerate(key_vals):
        for tb in range(GT):
            relcol = np.where(half_sel, rel[2 * tb + 1], rel[2 * tb])[:, None]
            dm = c_idx + relcol - P * tb - p_idx
            pj[:, ki * GW + tb * win : ki * GW + (tb + 1) * win] = np.where(
                (dm >= dlo) & (dm <= dhi), prior[dm + N - 1] * scale, np.float32(0.0)
            )
    # j0 tail: in-window relative offsets, per tile slot
    pj[:, n_pat * GW :] = np.tile(np.arange(win, dtype=np.float32), GT)[None, :]

    sumj_all = float(N * (N - 1) // 2)
    c1 = np.zeros((P, NT), np.float32)
    wsm = np.zeros((P, NT), np.float32)
    ii = np.zeros((P, NT), np.float32)
    for t in range(NT):
        wsa, wsb = ws2[2 * t], ws2[2 * t + 1]
        wsv = np.where(half_sel, float(wsb), float(wsa))
        c1[:, t] = sumj_all - (win * wsv + win * (win - 1) // 2)
        wsm[:, t] = wsv
        ii[:, t] = t * P + np.arange(P)

    # f32 consts: c1 | wsm | ii | 8*bq | 8*bk  (weights ship pre-scaled x8,
    # so the bias folded into the eviction must match)
    cst = np.ascontiguousarray(
        np.concatenate(
            [
                c1,
                wsm,
                ii,
                np.full((P, 1), WSCALE, np.float32) * bq.reshape(P, 1),
                np.full((P, 1), WSCALE, np.float32) * bk.reshape(P, 1),
            ],
            axis=1,
        ).astype(np.float32)
    )
    pj16 = np.ascontiguousarray(pj.astype(bf))

    # weights: wq chunks then wk chunks, [P, 4*MD], fp8 at x8 scale
    wq_h = (Wq * WSCALE).reshape(DCH, P, MD).transpose(1, 0, 2).reshape(P, DCH * MD)
    wk_h = (Wk * WSCALE).reshape(DCH, P, MD).transpose(1, 0, 2).reshape(P, DCH * MD)
    w2_h = np.ascontiguousarray(np.concatenate([wq_h, wk_h], axis=1)).astype(f8)

    in_maps = []
    for core in range(NCORES):
        xb = x[core]  # [N, D]
        # xt[p, c*N + j] = x[j, c*128 + p]
        xt_h = np.ascontiguousarray(
            xb.T.reshape(DCH, P, N).transpose(1, 0, 2).reshape(P, DCH * N)
        ).astype(f8)
        in_maps.append({"xt": xt_h, "w2": w2_h, "cst": cst, "pj": pj16})

    res = run_bass_kernel_spmd(nc, in_maps, list(range(NCORES)))
    last_run = (nc, in_maps)
    # y[p, t] = out[128t + p]  ->  out = y.T.flatten()
    out = np.stack(
        [res.results[c]["y"].T.reshape(-1) for c in range(NCORES)], axis=0
    )
    return out.astype(np.float32)


# revision 12
# speedup vs baseline: 1.0735x; 1.0206x over previous
"""Trainium2 Bass kernel for nn_DistanceLayer (gaussian-prior distance attention).

Math: out[b,i] = sum_j softmax_j(q_i.k_j * MD^-0.5 * prior(j-i))[j] * (j-i)

The gaussian prior (std=1) underflows so fast in f32 that outside a small
band |j-i| <= 7 the f32 score is exactly 0, so exp(score) is exactly 1.0.
Each softmax row is a small band of "interesting" values plus a uniform
far field with closed-form sums:

    T0_i = (N - win) + sum_window exp(s)            (denominator)
    T1_i = C1_i + sum_window exp(s)*jrel + ws_i * sum_window exp(s)
    out_i = T1_i / T0_i - i

where C1_i = sum_all_j j - sum_window_i j (exact ints in f32) and ws_i is
the window start of row i's 64-row half.  In-window far entries have
score exactly 0 (prior pattern is 0 outside the band) and contribute
exp(0)=1, which the constants account for.

v3 highlights (31.6us v1 -> 30.1us v2 -> this):
  - fp8e4m3 x/weights with DoubleRow matmuls: one PE instruction per
    (chunk, projection) at 0.5 cycles/row — projections cost half the
    PE columns of the bf16 version.
  - q and k of a chunk land in one [P, 1024] PSUM pair and evict in a
    single op (8 evictions -> 4), split ACT/DVE.
  - band groups sized (2,2,4,4,2,2): the first group only needs k chunk
    0 so postprocessing starts earlier, and the last group's serial
    tail (sp-mul -> exp -> ej -> reduce -> combine -> DMA) is short.
  - exp output and e*jrel share one tile so ONE reduce per group
    produces both window sums, written interleaved (col 2t = sum_e,
    2t+1 = sum_ec) so combine reads stride-2 views.
  - input DMAs split over gpsimd/ACT/SP queues; PE warmup starts at
    body start off a DVE memset so the 0.65->2.4GHz ramp completes
    early.

Sharding: pure data-parallel over batch B=8 across the 8 cores.
"""

import sys

sys.path.insert(0, "/opt/trn_rl_repo")

import ml_dtypes
import numpy as np

import concourse.bacc as bacc
import concourse.tile as tile
from concourse import mybir
from concourse.bass_utils import run_bass_kernel_spmd

B, N, D, MD = 8, 2048, 256, 128
NCORES = 8
P = 128
HR = P // 2  # 64-row halves
NT = N // P  # 16 row tiles
GROUPS = (2, 2, 4, 4, 2, 2)  # band group sizes in tiles
DCH = D // P  # 2 contraction chunks
PROJ_CHUNK = 512
NPC = N // PROJ_CHUNK  # 4 projection column chunks
PI = 3.1415926  # matches reference
WSCALE = 8.0  # fp8 weight pre-scale; pattern divides the x64 back out
F32 = mybir.dt.float32
BF16 = mybir.dt.bfloat16
F8 = mybir.dt.float8e4

_cache = {}
# exposed for test harness profiling: (nc, in_maps)
last_run = None


def _plan_band(prior_mean, prior_std):
    """f32 prior over every offset, exactly as the reference computes it,
    and the band of offsets whose scores can round exp() away from 1.0."""
    d = np.arange(-(N - 1), N, dtype=np.float32)
    ps = np.float32(prior_std)
    pm = np.float32(prior_mean)
    prior = (
        np.float32(1.0)
        / ps
        / np.sqrt(np.float32(2.0) * np.float32(PI))
        * np.exp(np.float32(-0.5) * (d - pm) ** 2 / ps**2)
    ).astype(np.float32)
    # |score| <= |prior| * |q.k*scale| ; bound the latter by 1024 (actual
    # max is ~7 for these glorot inputs).  exp(x) rounds to 1.0f for
    # |x| < 2^-26; use 2^-27 for margin.
    sig = np.abs(prior) * 1024.0 >= 2.0**-27
    if not sig.any():
        dlo, dhi = 0, 0
    else:
        dlo = int(d[sig].min())
        dhi = int(d[sig].max())
    return prior, dlo, dhi


def _window_geometry(dlo, dhi):
    """Per-64-row-half window starts ws2[32] plus deduplicated per-group
    prior patterns.  Pattern key for a group is (gt, rel offsets...) of
    its half-windows relative to the group's base row."""
    span = dhi - dlo
    win = HR + span + 1
    win = max(80, ((win + 15) // 16) * 16)
    assert win <= 448, f"prior band too wide for banded kernel: {dlo}..{dhi}"
    extra = win - (HR + span)
    ws2 = []
    for h in range(2 * NT):
        ws = min(max(h * HR + dlo - extra // 2, 0), N - win)
        lo_need = max(0, h * HR + dlo)
        hi_need = min(N - 1, h * HR + HR - 1 + dhi)
        assert ws <= lo_need and hi_need < ws + win, (h, ws, lo_need, hi_need)
        ws2.append(ws)
    gkeys = []
    t0 = 0
    for gt in GROUPS:
        base = t0 * P
        gkeys.append((gt,) + tuple(ws2[2 * t0 + i] - base for i in range(2 * gt)))
        t0 += gt
    key_vals = sorted(set(gkeys))
    key_idx = [key_vals.index(k) for k in gkeys]
    # pattern column offset of each key in the pj const
    key_off = {}
    off = 0
    for k in key_vals:
        key_off[k] = off
        off += k[0] * win
    return win, ws2, key_vals, key_idx, key_off, off


def _build(win, ws2, key_idx, key_off_list, pat_cols, use_bias):
    nc = bacc.Bacc()
    GWMAX = max(GROUPS) * win

    # f32 consts: c1 | wsm | ii | bq8 | bk8
    O_C1, O_WS, O_II, O_BQ = 0, NT, 2 * NT, 3 * NT
    CW = 3 * NT + 2
    # bf16 consts: patterns then j0
    PJW = pat_cols + GWMAX
    O_J0 = pat_cols

    xt_d = nc.dram_tensor("xt", [P, DCH * N], F8, kind="ExternalInput")
    w2_d = nc.dram_tensor("w2", [P, 2 * DCH * MD], F8, kind="ExternalInput")
    cs_d = nc.dram_tensor("cst", [P, CW], F32, kind="ExternalInput")
    pj_d = nc.dram_tensor("pj", [P, PJW], BF16, kind="ExternalInput")
    y_d = nc.dram_tensor("y", [P, NT], F32, kind="ExternalOutput")

    with tile.TileContext(nc) as tc:
        with (
            tc.tile_pool(name="const", bufs=1) as const,
            tc.tile_pool(name="psum_warm", bufs=1, space="PSUM") as psum_warm,
            tc.tile_pool(name="psum_proj", bufs=2, space="PSUM") as psum_proj,
            tc.tile_pool(name="psum_band", bufs=3, space="PSUM") as psum_band,
            tc.tile_pool(name="band_sp", bufs=2) as sp_pool,
            tc.tile_pool(name="band_ee", bufs=2) as ee_pool,
            tc.tile_pool(name="comb", bufs=1) as comb,
        ):
            # ---- engine warmups ----
            # PE: junk matmuls flip the HAM clock gate (full speed needs
            # ~3us of continuous busy) while the input DMAs land.  ACT:
            # one tiny Exp pulls the 1.3us ACT_TABLE_LOAD off the
            # critical path.  The memset runs on DVE at body start so the
            # ramp starts as early as possible.
            wtile = const.tile([P, 256], BF16, tag="warm_w")
            nc.vector.memset(wtile, 0.0)
            wact_in = const.tile([P, 1], F32, tag="warm_a")
            nc.vector.memset(wact_in, 0.0)
            wact_out = const.tile([P, 1], F32, tag="warm_ao")
            nc.scalar.activation(
                out=wact_out, in_=wact_in, func=mybir.ActivationFunctionType.Exp
            )

            # ---- input DMAs across 3 queues ----
            # chunk n4 needs both c-halves of its j range: A/B unlock
            # chunks 0-1, C/D chunks 2-3
            w2_s = const.tile([P, 2 * DCH * MD], F8, tag="w2")
            nc.sync.dma_start(out=w2_s, in_=w2_d[:, :])
            xt_s = const.tile([P, DCH * N], F8, tag="xt")
            half = N // 2
            nc.gpsimd.dma_start(out=xt_s[:, 0:half], in_=xt_d[:, 0:half])
            nc.scalar.dma_start(out=xt_s[:, N : N + half], in_=xt_d[:, N : N + half])
            nc.sync.dma_start(out=xt_s[:, half:N], in_=xt_d[:, half:N])
            nc.gpsimd.dma_start(
                out=xt_s[:, N + half : 2 * N], in_=xt_d[:, N + half : 2 * N]
            )
            pj_s = const.tile([P, PJW], BF16, tag="pj")
            nc.sync.dma_start(out=pj_s, in_=pj_d[:, :])
            cs_s = const.tile([P, CW], F32, tag="cst")
            nc.gpsimd.dma_start(out=cs_s, in_=cs_d[:, :])

            wps = psum_warm.tile([P, 256], F32, tag="warm")
            for _ in range(5):
                nc.tensor.matmul(
                    wps, lhsT=wtile[:, :P], rhs=wtile, start=True, stop=True
                )

            qkT = const.tile([P, 2 * N], BF16, tag="qkT")  # q | k
            # interleaved sums: col 2t = sum_e[t], 2t+1 = sum_ec[t]
            sums = const.tile([P, 2 * NT], BF16, tag="sums")

            # ---- projection chunk: q and k into one [P, 1024] psum pair,
            # each a single fp8 DoubleRow matmul (contraction pairs are the
            # two D-halves), then ONE fused eviction ----
            EVICT_ENG = ["act", "dve", "act", "dve"]

            def emit_proj(n4):
                ps_t = psum_proj.tile([P, 2 * PROJ_CHUNK], F32, tag="proj")
                rhs3 = xt_s[:].rearrange("p (c j) -> p c j", c=DCH)[
                    :, :, n4 * PROJ_CHUNK : (n4 + 1) * PROJ_CHUNK
                ]
                for pj in range(2):  # 0=q, 1=k
                    lhsT3 = w2_s[
                        :, 2 * pj * MD : (2 * pj + 2) * MD
                    ].rearrange("p (c m) -> p c m", c=DCH)
                    nc.tensor.matmul(
                        ps_t[:, pj * PROJ_CHUNK : (pj + 1) * PROJ_CHUNK],
                        lhsT=lhsT3,
                        rhs=rhs3,
                        start=True,
                        stop=True,
                        perf_mode=mybir.MatmulPerfMode.DoubleRow,
                    )
                # fused eviction: [P, 2, 512] view of qkT at (q, k) slices
                dst = qkT[:].rearrange("p (s j) -> p s j", s=2)[
                    :, :, n4 * PROJ_CHUNK : (n4 + 1) * PROJ_CHUNK
                ]
                eng = EVICT_ENG[n4]
                if use_bias:
                    # per-partition bias differs for q and k: two ops
                    for pj in range(2):
                        b_s = cs_s[:, O_BQ + pj : O_BQ + pj + 1]
                        d1 = qkT[:, pj * N + n4 * PROJ_CHUNK : pj * N + (n4 + 1) * PROJ_CHUNK]
                        s1 = ps_t[:, pj * PROJ_CHUNK : (pj + 1) * PROJ_CHUNK]
                        if eng == "act":
                            nc.scalar.activation(
                                out=d1, in_=s1,
                                func=mybir.ActivationFunctionType.Identity,
                                bias=b_s, scale=1.0,
                            )
                        else:
                            nc.vector.tensor_scalar_add(d1, s1, b_s)
                else:
                    if eng == "act":
                        nc.scalar.copy(out=dst, in_=ps_t)
                    else:
                        nc.vector.tensor_copy(dst, ps_t)

            # ---- band group ----
            def emit_group(g, t0, gt):
                gw = gt * win
                ps_full = psum_band.tile([P, GWMAX], F32, tag="band")
                ps_s = ps_full[:, :gw]
                for tb in range(gt):
                    t = t0 + tb
                    for hb in range(2):
                        ws = ws2[2 * t + hb]
                        nc.tensor.matmul(
                            ps_s[hb * HR : (hb + 1) * HR, tb * win : (tb + 1) * win],
                            lhsT=qkT[:, t * P + hb * HR : t * P + (hb + 1) * HR],
                            rhs=qkT[:, N + ws : N + ws + win],
                            start=True,
                            stop=True,
                        )
                pat = pj_s[:, key_off_list[g] : key_off_list[g] + gw]
                sp_full = sp_pool.tile([P, GWMAX], BF16, tag="sp")
                sp_t = sp_full[:, :gw]
                nc.vector.tensor_mul(sp_t, ps_s, pat)
                ee_full = ee_pool.tile([P, 2 * GWMAX], BF16, tag="ee")
                ee_t = ee_full[:, : 2 * gw]
                nc.scalar.activation(
                    out=ee_t[:, :gw], in_=sp_t,
                    func=mybir.ActivationFunctionType.Exp,
                )
                nc.vector.tensor_mul(
                    ee_t[:, gw : 2 * gw], ee_t[:, :gw], pj_s[:, O_J0 : O_J0 + gw]
                )
                # one reduce for both sums; out cols interleave as
                # (kind, tile) -> 2*(t0+tb)+kind via a [2, gt] out view
                out_ap = sums[:, 2 * t0 : 2 * (t0 + gt)].rearrange(
                    "p (t k) -> p k t", k=2
                )
                with nc.allow_low_precision("bf16 window sums, ~1e-5 rel out err"):
                    nc.vector.tensor_reduce(
                        out=out_ap,
                        in_=ee_t.rearrange("p (t w) -> p t w", w=win),
                        axis=mybir.AxisListType.X,
                        op=mybir.AluOpType.add,
                    )

            # ---- combine: out = (c1 + sum_ec + ws*sum_e)/(N-win+sum_e) - i ----
            c1_s = cs_s[:, O_C1 : O_C1 + NT]
            ws_s = cs_s[:, O_WS : O_WS + NT]
            ii_s = cs_s[:, O_II : O_II + NT]
            outv2 = comb.tile([P, NT], F32, tag="outv2")

            def emit_combine(sl):
                w = sl.stop - sl.start
                se = sums[:, 2 * sl.start : 2 * sl.stop].rearrange(
                    "p (t k) -> p t k", k=2
                )[:, :, 0]
                sec = sums[:, 2 * sl.start : 2 * sl.stop].rearrange(
                    "p (t k) -> p t k", k=2
                )[:, :, 1]
                t0 = comb.tile([P, w], F32, tag="t0")
                nc.gpsimd.tensor_scalar_add(t0, se, float(N - win))
                rec = comb.tile([P, w], F32, tag="rec")
                nc.vector.reciprocal(rec, t0)
                tmp = comb.tile([P, w], F32, tag="tmp")
                nc.gpsimd.tensor_mul(tmp, ws_s[:, sl], se)
                num = comb.tile([P, w], F32, tag="num")
                nc.gpsimd.tensor_add(num, c1_s[:, sl], sec)
                num2 = comb.tile([P, w], F32, tag="num2")
                nc.gpsimd.tensor_add(num2, num, tmp)
                outv = comb.tile([P, w], F32, tag="outv")
                nc.vector.tensor_mul(outv, num2, rec)
                nc.gpsimd.tensor_sub(outv2[:, sl], outv, ii_s[:, sl])
                nc.sync.dma_start(out=y_d[:, sl], in_=outv2[:, sl])

            # group g's k-window tail crosses into the next proj chunk, so
            # group g unlocks after proj chunk ceil((last_ws+win)/512)-1
            gstart = np.cumsum([0] + list(GROUPS))[:-1]

            emit_proj(0)
            emit_group(0, 0, GROUPS[0])  # tiles 0-1: k cols < 512
            emit_proj(1)
            emit_group(1, int(gstart[1]), GROUPS[1])  # tiles 2-3: k < 1024
            emit_proj(2)
            emit_group(2, int(gstart[2]), GROUPS[2])  # tiles 4-7: k < 1546
            emit_proj(3)
            emit_combine(slice(0, 4))
            emit_group(3, int(gstart[3]), GROUPS[3])  # tiles 8-11
            emit_group(4, int(gstart[4]), GROUPS[4])  # tiles 12-13
            emit_combine(slice(4, 12))
            emit_group(5, int(gstart[5]), GROUPS[5])  # tiles 14-15
            emit_combine(slice(12, NT))

    nc.finalize()
    return nc


def kernel(x, Wq, bq, Wk, bk, prior_mean, prior_std):
    global last_run
    x = np.asarray(x, dtype=np.float32)
    Wq = np.asarray(Wq, dtype=np.float32)
    Wk = np.asarray(Wk, dtype=np.float32)
    bq = np.asarray(bq, dtype=np.float32)
    bk = np.asarray(bk, dtype=np.float32)

    prior, dlo, dhi = _plan_band(
        float(np.asarray(prior_mean)[0]), float(np.asarray(prior_std)[0])
    )
    win, ws2, key_vals, key_idx, key_off, pat_cols = _window_geometry(dlo, dhi)
    use_bias = bool(np.any(bq != 0.0) or np.any(bk != 0.0))
    key_off_list = [key_off[key_vals[key_idx[g]]] for g in range(len(GROUPS))]

    ckey = (win, tuple(ws2), tuple(key_idx), use_bias)
    if ckey not in _cache:
        _cache[ckey] = _build(win, ws2, key_idx, key_off_list, pat_cols, use_bias)
    nc = _cache[ckey]

    bf = ml_dtypes.bfloat16
    f8 = ml_dtypes.float8_e4m3
    scale = np.float32(MD**-0.5) / np.float32(WSCALE * WSCALE)
    GWMAX = max(GROUPS) * win

    # prior*scale patterns per distinct key, then j0
    p_idx = np.arange(P)[:, None]
    c_idx = np.arange(win)[None, :]
    half_sel = np.arange(P) >= HR
    pj = np.zeros((P, pat_cols + GWMAX), np.float32)
    for kv in key_vals:
        gt, rel = kv[0], kv[1:]
        off = key_off[kv]
        for tb in range(gt):
            relcol = np.where(half_sel, rel[2 * tb + 1], rel[2 * tb])[:, None]
            dm = c_idx + relcol - P * tb - p_idx
            pj[:, off + tb * win : off + (tb + 1) * win] = np.where(
                (dm >= dlo) & (dm <= dhi), prior[dm + N - 1] * scale, np.float32(0.0)
            )
    pj[:, pat_cols:] = np.tile(np.arange(win, dtype=np.float32), max(GROUPS))[None, :]

    sumj_all = float(N * (N - 1) // 2)
    c1 = np.zeros((P, NT), np.float32)
    wsm = np.zeros((P, NT), np.float32)
    ii = np.zeros((P, NT), np.float32)
    for t in range(NT):
        wsa, wsb = ws2[2 * t], ws2[2 * t + 1]
        wsv = np.where(half_sel, float(wsb), float(wsa))
        c1[:, t] = sumj_all - (win * wsv + win * (win - 1) // 2)
        wsm[:, t] = wsv
        ii[:, t] = t * P + np.arange(P)

    # f32 consts: c1 | wsm | ii | 8*bq | 8*bk  (weights ship pre-scaled x8,
    # so the bias folded into the eviction must match)
    cst = np.ascontiguousarray(
        np.concatenate(
            [
                c1,
                wsm,
                ii,
                np.float32(WSCALE) * bq.reshape(P, 1),
                np.float32(WSCALE) * bk.reshape(P, 1),
            ],
            axis=1,
        ).astype(np.float32)
    )
    pj16 = np.ascontiguousarray(pj.astype(bf))

    # weights: wq chunks then wk chunks, [P, 4*MD], fp8 at x8 scale
    wq_h = (Wq * WSCALE).reshape(DCH, P, MD).transpose(1, 0, 2).reshape(P, DCH * MD)
    wk_h = (Wk * WSCALE).reshape(DCH, P, MD).transpose(1, 0, 2).reshape(P, DCH * MD)
    w2_h = np.ascontiguousarray(np.concatenate([wq_h, wk_h], axis=1)).astype(f8)

    in_maps = []
    for core in range(NCORES):
        xb = x[core]  # [N, D]
        # xt[p, c*N + j] = x[j, c*128 + p]
        xt_h = np.ascontiguousarray(
            xb.T.reshape(DCH, P, N).transpose(1, 0, 2).reshape(P, DCH * N)
        ).astype(f8)
        in_maps.append({"xt": xt_h, "w2": w2_h, "cst": cst, "pj": pj16})

    res = run_bass_kernel_spmd(nc, in_maps, list(range(NCORES)))
    last_run = (nc, in_maps)
    # y[p, t] = out[128t + p]  ->  out = y.T.flatten()
    out = np.stack(
        [res.results[c]["y"].T.reshape(-1) for c in range(NCORES)], axis=0
    )
    return out.astype(np.float32)


# revision 14
# speedup vs baseline: 1.0763x; 1.0027x over previous
"""Trainium2 Bass kernel for nn_DistanceLayer (gaussian-prior distance attention).

Math: out[b,i] = sum_j softmax_j(q_i.k_j * MD^-0.5 * prior(j-i))[j] * (j-i)

The gaussian prior (std=1) underflows so fast in f32 that outside a small
band |j-i| <= 7 the f32 score is exactly 0, so exp(score) is exactly 1.0.
Each softmax row is a small band of "interesting" values plus a uniform
far field with closed-form sums:

    T0_i = (N - win) + sum_window exp(s)            (denominator)
    T1_i = C1_i + sum_window exp(s)*jrel + ws_i * sum_window exp(s)
    out_i = T1_i / T0_i - i

where C1_i = sum_all_j j - sum_window_i j (exact ints in f32) and ws_i is
the window start of row i's 32-row quarter.  In-window far entries have
score exactly 0 (prior pattern is 0 outside the band) and contribute
exp(0)=1, which the constants account for.

v4 layout/schedule:
  - 32-row QUARTER windows (win=48 vs 80 for 64-row halves): 40% fewer
    score elements, so the DVE-locked postprocessing (prior-mul and the
    window reduces, which have no 2x dtype mode) fits the engine budget.
    PE pays ~64 small band matmuls, but has DoubleRow headroom.
  - fp8e4m3 x/weights with DoubleRow matmuls: one PE instruction per
    (chunk, projection) at 0.5 cycles/row.
  - q and k of a chunk share one [P, 2, 512] PSUM pair and evict in a
    single fused op (ACT-heavy split).
  - band groups of (3,4,4,3,2) tiles: group 0 only needs k chunk 0 so
    postprocessing starts at the first eviction, and the last group's
    serial tail is short.  exp output and e*jrel share one tile; ONE
    reduce per group yields both sums interleaved (col 2t / 2t+1).
  - input DMAs ordered for earliest projection unlock: sync (xtA, xtC,
    patterns), scalar (w2, xtB, xtD), gpsimd (combine consts); PE junk
    matmuls ramp the 0.65->2.4GHz clock until real data lands.

Sharding: pure data-parallel over batch B=8 across the 8 cores.
"""

import sys

sys.path.insert(0, "/opt/trn_rl_repo")

import ml_dtypes
import numpy as np

import concourse.bacc as bacc
import concourse.tile as tile
from concourse import mybir
from concourse.bass_utils import run_bass_kernel_spmd

B, N, D, MD = 8, 2048, 256, 128
NCORES = 8
P = 128
QR = 32  # quarter height
NQ = N // QR  # 64 quarters
NT = N // P  # 16 row tiles
GROUPS = (3, 4, 4, 3, 2)  # band group sizes in tiles
GSTART = (0, 3, 7, 11, 14)
# group g's k windows reach into proj chunk UNLOCK[g]; emitted after it
UNLOCK = (0, 1, 2, 3, 3)
DCH = D // P  # 2 contraction chunks
PROJ_CHUNK = 512
NPC = N // PROJ_CHUNK  # 4 projection column chunks
PI = 3.1415926  # matches reference
WSCALE = 8.0  # fp8 weight pre-scale; pattern divides the x64 back out
F32 = mybir.dt.float32
BF16 = mybir.dt.bfloat16
F8 = mybir.dt.float8e4

_cache = {}
# exposed for test harness profiling: (nc, in_maps)
last_run = None


def _plan_band(prior_mean, prior_std):
    """f32 prior over every offset, exactly as the reference computes it,
    and the band of offsets whose scores can round exp() away from 1.0."""
    d = np.arange(-(N - 1), N, dtype=np.float32)
    ps = np.float32(prior_std)
    pm = np.float32(prior_mean)
    prior = (
        np.float32(1.0)
        / ps
        / np.sqrt(np.float32(2.0) * np.float32(PI))
        * np.exp(np.float32(-0.5) * (d - pm) ** 2 / ps**2)
    ).astype(np.float32)
    # |score| <= |prior| * |q.k*scale| ; bound the latter by 1024 (actual
    # max is ~7 for these glorot inputs).  exp(x) rounds to 1.0f for
    # |x| < 2^-26; use 2^-27 for margin.
    sig = np.abs(prior) * 1024.0 >= 2.0**-27
    if not sig.any():
        dlo, dhi = 0, 0
    else:
        dlo = int(d[sig].min())
        dhi = int(d[sig].max())
    return prior, dlo, dhi


def _window_geometry(dlo, dhi):
    """Per-quarter window starts ws4[64] plus deduplicated per-group
    prior patterns.  Pattern key for a group is (gt, rel offsets...) of
    its quarter-windows relative to the group's base row."""
    span = dhi - dlo
    win = QR + span + 1
    win = max(48, ((win + 15) // 16) * 16)
    assert win <= 192, f"prior band too wide for quarter-banded kernel: {dlo}..{dhi}"
    extra = win - (QR + span)
    ws4 = []
    for h in range(NQ):
        ws = min(max(h * QR + dlo - extra // 2, 0), N - win)
        lo_need = max(0, h * QR + dlo)
        hi_need = min(N - 1, h * QR + QR - 1 + dhi)
        assert ws <= lo_need and hi_need < ws + win, (h, ws, lo_need, hi_need)
        ws4.append(ws)
    gkeys = []
    for g, gt in enumerate(GROUPS):
        t0 = GSTART[g]
        base = t0 * P
        gkeys.append((gt,) + tuple(ws4[4 * t0 + i] - base for i in range(4 * gt)))
    key_vals = sorted(set(gkeys))
    key_idx = [key_vals.index(k) for k in gkeys]
    key_off = {}
    off = 0
    for k in key_vals:
        key_off[k] = off
        off += k[0] * win
    return win, ws4, key_vals, key_idx, key_off, off


def _build(win, ws4, key_idx, key_off_list, pat_cols, use_bias):
    nc = bacc.Bacc()
    GWMAX = max(GROUPS) * win

    # f32 consts: c1 | wsm | ii | bq8 | bk8
    O_C1, O_WS, O_II, O_BQ = 0, NT, 2 * NT, 3 * NT
    CW = 3 * NT + 2
    # bf16 consts: patterns then j0
    PJW = pat_cols + GWMAX
    O_J0 = pat_cols

    xt_d = nc.dram_tensor("xt", [P, DCH * N], F8, kind="ExternalInput")
    w2_d = nc.dram_tensor("w2", [P, 2 * DCH * MD], F8, kind="ExternalInput")
    cs_d = nc.dram_tensor("cst", [P, CW], F32, kind="ExternalInput")
    pj_d = nc.dram_tensor("pj", [P, PJW], BF16, kind="ExternalInput")
    y_d = nc.dram_tensor("y", [P, NT], F32, kind="ExternalOutput")

    with tile.TileContext(nc) as tc:
        with (
            tc.tile_pool(name="const", bufs=1) as const,
            tc.tile_pool(name="psum_proj", bufs=3, space="PSUM") as psum_proj,
            tc.tile_pool(name="psum_band", bufs=2, space="PSUM") as psum_band,
            tc.tile_pool(name="band_sp", bufs=2) as sp_pool,
            tc.tile_pool(name="band_ee", bufs=2) as ee_pool,
            tc.tile_pool(name="comb", bufs=1) as comb,
        ):
            # ---- input DMAs first: earliest descriptor generation ----
            # first projection chunk needs w2 + xtA + xtB; queues ordered
            # so those land first.
            w2_s = const.tile([P, 2 * DCH * MD], F8, tag="w2")
            xt_s = const.tile([P, DCH * N], F8, tag="xt")
            pj_s = const.tile([P, PJW], BF16, tag="pj")
            cs_s = const.tile([P, CW], F32, tag="cst")
            half = N // 2
            nc.sync.dma_start(out=xt_s[:, 0:half], in_=xt_d[:, 0:half])  # A
            nc.scalar.dma_start(out=w2_s, in_=w2_d[:, :])
            nc.scalar.dma_start(  # B
                out=xt_s[:, N : N + half], in_=xt_d[:, N : N + half]
            )
            nc.sync.dma_start(out=xt_s[:, half:N], in_=xt_d[:, half:N])  # C
            nc.scalar.dma_start(  # D
                out=xt_s[:, N + half : 2 * N], in_=xt_d[:, N + half : 2 * N]
            )
            nc.sync.dma_start(out=pj_s, in_=pj_d[:, :])
            nc.gpsimd.dma_start(out=cs_s, in_=cs_d[:, :])

            # ---- engine warmups ----
            # PE: junk matmuls flip the HAM clock gate (full speed needs
            # ~3us of continuous busy) while the input DMAs land.  ACT:
            # one tiny Exp pulls the 1.3us ACT_TABLE_LOAD off the
            # critical path.
            wtile = const.tile([P, GWMAX], BF16, tag="warm_w")
            nc.vector.memset(wtile, 0.0)
            wact_in = const.tile([P, 1], F32, tag="warm_a")
            nc.vector.memset(wact_in, 0.0)
            wact_out = const.tile([P, 1], F32, tag="warm_ao")
            nc.scalar.activation(
                out=wact_out, in_=wact_in, func=mybir.ActivationFunctionType.Exp
            )
            for _ in range(7):
                wps = psum_band.tile([P, GWMAX], F32, tag="band")
                nc.tensor.matmul(
                    wps, lhsT=wtile[:, :P], rhs=wtile, start=True, stop=True
                )

            qkT = const.tile([P, 2 * N], BF16, tag="qkT")  # q | k
            # interleaved sums: col 2t = sum_e[t], 2t+1 = sum_ec[t]
            sums = const.tile([P, 2 * NT], BF16, tag="sums")

            # ---- projection chunk: q and k into one [P, 1024] psum pair,
            # each a single fp8 DoubleRow matmul (contraction pairs are the
            # two D-halves), then ONE fused eviction ----
            EVICT_ENG = ["act", "dve", "act", "act"]

            def emit_proj(n4):
                ps_t = psum_proj.tile([P, 2 * PROJ_CHUNK], F32, tag="proj")
                rhs3 = xt_s[:].rearrange("p (c j) -> p c j", c=DCH)[
                    :, :, n4 * PROJ_CHUNK : (n4 + 1) * PROJ_CHUNK
                ]
                for pj in range(2):  # 0=q, 1=k
                    lhsT3 = w2_s[
                        :, 2 * pj * MD : (2 * pj + 2) * MD
                    ].rearrange("p (c m) -> p c m", c=DCH)
                    nc.tensor.matmul(
                        ps_t[:, pj * PROJ_CHUNK : (pj + 1) * PROJ_CHUNK],
                        lhsT=lhsT3,
                        rhs=rhs3,
                        start=True,
                        stop=True,
                        perf_mode=mybir.MatmulPerfMode.DoubleRow,
                    )
                # fused eviction: [P, 2, 512] view of qkT at (q, k) slices
                dst = qkT[:].rearrange("p (s j) -> p s j", s=2)[
                    :, :, n4 * PROJ_CHUNK : (n4 + 1) * PROJ_CHUNK
                ]
                src = ps_t[:].rearrange("p (s j) -> p s j", s=2)
                eng = EVICT_ENG[n4]
                if use_bias:
                    # per-partition bias differs for q and k: two ops
                    for pj in range(2):
                        b_s = cs_s[:, O_BQ + pj : O_BQ + pj + 1]
                        d1 = qkT[:, pj * N + n4 * PROJ_CHUNK : pj * N + (n4 + 1) * PROJ_CHUNK]
                        s1 = ps_t[:, pj * PROJ_CHUNK : (pj + 1) * PROJ_CHUNK]
                        if eng == "act":
                            nc.scalar.activation(
                                out=d1, in_=s1,
                                func=mybir.ActivationFunctionType.Identity,
                                bias=b_s, scale=1.0,
                            )
                        else:
                            nc.vector.tensor_scalar_add(d1, s1, b_s)
                else:
                    if eng == "act":
                        nc.scalar.copy(out=dst, in_=src)
                    else:
                        nc.vector.tensor_copy(dst, src)

            # ---- band group: 4*gt quarter matmuls, postproc in one pass ----
            def emit_group(g):
                t0, gt = GSTART[g], GROUPS[g]
                gw = gt * win
                ps_full = psum_band.tile([P, GWMAX], F32, tag="band")
                ps_s = ps_full[:, :gw]
                for tb in range(gt):
                    t = t0 + tb
                    for qd in range(4):
                        ws = ws4[4 * t + qd]
                        nc.tensor.matmul(
                            ps_s[qd * QR : (qd + 1) * QR, tb * win : (tb + 1) * win],
                            lhsT=qkT[:, t * P + qd * QR : t * P + (qd + 1) * QR],
                            rhs=qkT[:, N + ws : N + ws + win],
                            start=True,
                            stop=True,
                            tile_position=(0, qd * QR),
                        )
                pat = pj_s[:, key_off_list[g] : key_off_list[g] + gw]
                sp_full = sp_pool.tile([P, GWMAX], BF16, tag="sp")
                sp_t = sp_full[:, :gw]
                nc.vector.tensor_mul(sp_t, ps_s, pat)
                ee_full = ee_pool.tile([P, 2 * GWMAX], BF16, tag="ee")
                ee_t = ee_full[:, : 2 * gw]
                nc.scalar.activation(
                    out=ee_t[:, :gw], in_=sp_t,
                    func=mybir.ActivationFunctionType.Exp,
                )
                nc.gpsimd.tensor_mul(
                    ee_t[:, gw : 2 * gw], ee_t[:, :gw], pj_s[:, O_J0 : O_J0 + gw]
                )
                # one reduce for both sums; out cols interleave as
                # (kind, tile) -> 2*(t0+tb)+kind via a [2, gt] out view
                out_ap = sums[:, 2 * t0 : 2 * (t0 + gt)].rearrange(
                    "p (t k) -> p k t", k=2
                )
                # bf16 sums: worst-case 0.4% of ~8e3 is ~0.02 abs on a
                # +-1023-scale output (tolerance 2e-2 rel) — safe.
                with nc.allow_low_precision("bf16 window sums, ~1e-5 rel out err"):
                    nc.vector.tensor_reduce(
                        out=out_ap,
                        in_=ee_t.rearrange("p (t w) -> p t w", w=win),
                        axis=mybir.AxisListType.X,
                        op=mybir.AluOpType.add,
                    )

            # ---- combine: out = (c1 + sum_ec + ws*sum_e)/(N-win+sum_e) - i ----
            c1_s = cs_s[:, O_C1 : O_C1 + NT]
            ws_s = cs_s[:, O_WS : O_WS + NT]
            ii_s = cs_s[:, O_II : O_II + NT]
            outv2 = comb.tile([P, NT], F32, tag="outv2")

            def emit_combine(sl):
                w = sl.stop - sl.start
                se = sums[:, 2 * sl.start : 2 * sl.stop].rearrange(
                    "p (t k) -> p t k", k=2
                )[:, :, 0]
                sec = sums[:, 2 * sl.start : 2 * sl.stop].rearrange(
                    "p (t k) -> p t k", k=2
                )[:, :, 1]
                t0 = comb.tile([P, w], F32, tag="t0")
                nc.gpsimd.tensor_scalar_add(t0, se, float(N - win))
                rec = comb.tile([P, w], F32, tag="rec")
                nc.vector.reciprocal(rec, t0)
                tmp = comb.tile([P, w], F32, tag="tmp")
                nc.gpsimd.tensor_mul(tmp, ws_s[:, sl], se)
                num = comb.tile([P, w], F32, tag="num")
                nc.gpsimd.tensor_add(num, c1_s[:, sl], sec)
                num2 = comb.tile([P, w], F32, tag="num2")
                nc.gpsimd.tensor_add(num2, num, tmp)
                outv = comb.tile([P, w], F32, tag="outv")
                nc.vector.tensor_mul(outv, num2, rec)
                nc.gpsimd.tensor_sub(outv2[:, sl], outv, ii_s[:, sl])
                nc.sync.dma_start(out=y_d[:, sl], in_=outv2[:, sl])

            emit_proj(0)
            emit_group(0)  # tiles 0-2, k cols < 512
            emit_proj(1)
            emit_group(1)  # tiles 3-6, k < 1024
            emit_proj(2)
            emit_group(2)  # tiles 7-10, k < 1536
            emit_proj(3)
            emit_combine(slice(0, 7))
            emit_group(3)  # tiles 11-13
            emit_group(4)  # tiles 14-15
            emit_combine(slice(7, 14))
            emit_combine(slice(14, NT))

    nc.finalize()
    return nc


def kernel(x, Wq, bq, Wk, bk, prior_mean, prior_std):
    global last_run
    x = np.asarray(x, dtype=np.float32)
    Wq = np.asarray(Wq, dtype=np.float32)
    Wk = np.asarray(Wk, dtype=np.float32)
    bq = np.asarray(bq, dtype=np.float32)
    bk = np.asarray(bk, dtype=np.float32)

    prior, dlo, dhi = _plan_band(
        float(np.asarray(prior_mean)[0]), float(np.asarray(prior_std)[0])
    )
    win, ws4, key_vals, key_idx, key_off, pat_cols = _window_geometry(dlo, dhi)
    use_bias = bool(np.any(bq != 0.0) or np.any(bk != 0.0))
    key_off_list = [key_off[key_vals[key_idx[g]]] for g in range(len(GROUPS))]

    ckey = (win, tuple(ws4), tuple(key_idx), use_bias)
    if ckey not in _cache:
        _cache[ckey] = _build(win, ws4, key_idx, key_off_list, pat_cols, use_bias)
    nc = _cache[ckey]

    bf = ml_dtypes.bfloat16
    f8 = ml_dtypes.float8_e4m3
    scale = np.float32(MD**-0.5) / np.float32(WSCALE * WSCALE)
    GWMAX = max(GROUPS) * win

    # prior*scale patterns per distinct key, then j0
    p_idx = np.arange(P)[:, None]
    c_idx = np.arange(win)[None, :]
    quad = np.arange(P) // QR  # quarter index of each partition
    pj = np.zeros((P, pat_cols + GWMAX), np.float32)
    for kv in key_vals:
        gt, rel = kv[0], kv[1:]
        off = key_off[kv]
        for tb in range(gt):
            relcol = np.asarray(rel)[4 * tb + quad][:, None]
            dm = c_idx + relcol - P * tb - p_idx
            pj[:, off + tb * win : off + (tb + 1) * win] = np.where(
                (dm >= dlo) & (dm <= dhi), prior[dm + N - 1] * scale, np.float32(0.0)
            )
    pj[:, pat_cols:] = np.tile(np.arange(win, dtype=np.float32), max(GROUPS))[None, :]

    sumj_all = float(N * (N - 1) // 2)
    c1 = np.zeros((P, NT), np.float32)
    wsm = np.zeros((P, NT), np.float32)
    ii = np.zeros((P, NT), np.float32)
    ws4a = np.asarray(ws4, np.float32)
    for t in range(NT):
        wsv = ws4a[4 * t + quad]
        c1[:, t] = sumj_all - (win * wsv + win * (win - 1) // 2)
        wsm[:, t] = wsv
        ii[:, t] = t * P + np.arange(P)

    # f32 consts: c1 | wsm | ii | 8*bq | 8*bk  (weights ship pre-scaled x8,
    # so the bias folded into the eviction must match)
    cst = np.ascontiguousarray(
        np.concatenate(
            [
                c1,
                wsm,
                ii,
                np.float32(WSCALE) * bq.reshape(P, 1),
                np.float32(WSCALE) * bk.reshape(P, 1),
            ],
            axis=1,
        ).astype(np.float32)
    )
    pj16 = np.ascontiguousarray(pj.astype(bf))

    # weights: wq chunks then wk chunks, [P, 4*MD], fp8 at x8 scale
    wq_h = (Wq * WSCALE).reshape(DCH, P, MD).transpose(1, 0, 2).reshape(P, DCH * MD)
    wk_h = (Wk * WSCALE).reshape(DCH, P, MD).transpose(1, 0, 2).reshape(P, DCH * MD)
    w2_h = np.ascontiguousarray(np.concatenate([wq_h, wk_h], axis=1)).astype(f8)

    in_maps = []
    for core in range(NCORES):
        xb = x[core]  # [N, D]
        # xt[p, c*N + j] = x[j, c*128 + p]
        xt_h = np.ascontiguousarray(
            xb.T.reshape(DCH, P, N).transpose(1, 0, 2).reshape(P, DCH * N)
        ).astype(f8)
        in_maps.append({"xt": xt_h, "w2": w2_h, "cst": cst, "pj": pj16})

    res = run_bass_kernel_spmd(nc, in_maps, list(range(NCORES)))
    last_run = (nc, in_maps)
    # y[p, t] = out[128t + p]  ->  out = y.T.flatten()
    out = np.stack(
        [res.results[c]["y"].T.reshape(-1) for c in range(NCORES)], axis=0
    )
    return out.astype(np.float32)


# revision 19
# speedup vs baseline: 1.1550x; 1.0731x over previous
"""Trainium2 Bass kernel for nn_DistanceLayer (gaussian-prior distance attention).

Math: out[b,i] = sum_j softmax_j(q_i.k_j * MD^-0.5 * prior(j-i))[j] * (j-i)

The gaussian prior (std=1) underflows so fast in f32 that outside a small
band |j-i| <= 7 the f32 score is exactly 0, so exp(score) is exactly 1.0.
Each softmax row is a small band of "interesting" values plus a uniform
far field with closed-form sums:

    T0_i = (N - win) + sum_window exp(s)            (denominator)
    T1_i = C1_i + sum_window exp(s)*jrel + ws_i * sum_window exp(s)
    out_i = T1_i / T0_i - i

where C1_i = sum_all_j j - sum_window_i j (exact ints in f32) and ws_i is
the window start of row i's 32-row quarter.  In-window far entries have
score exactly 0 (prior pattern is 0 outside the band) and contribute
exp(0)=1, which the constants account for.

v4 layout/schedule:
  - 32-row QUARTER windows (win=48 vs 80 for 64-row halves): 40% fewer
    score elements, so the DVE-locked postprocessing (prior-mul and the
    window reduces, which have no 2x dtype mode) fits the engine budget.
    PE pays ~64 small band matmuls, but has DoubleRow headroom.
  - fp8e4m3 x/weights with DoubleRow matmuls: one PE instruction per
    (chunk, projection) at 0.5 cycles/row.
  - q and k of a chunk share one [P, 2, 512] PSUM pair and evict in a
    single fused op (ACT-heavy split).
  - band groups of (3,4,4,3,2) tiles: group 0 only needs k chunk 0 so
    postprocessing starts at the first eviction, and the last group's
    serial tail is short.  exp output and e*jrel share one tile; ONE
    reduce per group yields both sums interleaved (col 2t / 2t+1).
  - input DMAs ordered for earliest projection unlock: sync (xtA, xtC,
    patterns), scalar (w2, xtB, xtD), gpsimd (combine consts); PE junk
    matmuls ramp the 0.65->2.4GHz clock until real data lands.

Sharding: pure data-parallel over batch B=8 across the 8 cores.
"""

import sys

sys.path.insert(0, "/opt/trn_rl_repo")

import ml_dtypes
import numpy as np

import concourse.bacc as bacc
import concourse.tile as tile
from concourse import mybir
from concourse.bass_utils import run_bass_kernel_spmd

B, N, D, MD = 8, 2048, 256, 128
NCORES = 8
P = 128
QR = 32  # quarter height
NQ = N // QR  # 64 quarters
NT = N // P  # 16 row tiles
GROUPS = (3, 4, 4, 3, 2)  # band group sizes in tiles
GSTART = (0, 3, 7, 11, 14)
# group g's k windows reach into proj chunk UNLOCK[g]; emitted after it
UNLOCK = (0, 1, 2, 3, 3)
DCH = D // P  # 2 contraction chunks
PROJ_CHUNK = 512
NPC = N // PROJ_CHUNK  # 4 projection column chunks
PI = 3.1415926  # matches reference
WSCALE = 8.0  # fp8 weight pre-scale; pattern divides the x64 back out
F32 = mybir.dt.float32
BF16 = mybir.dt.bfloat16
F8 = mybir.dt.float8e4

_cache = {}
# exposed for test harness profiling: (nc, in_maps)
last_run = None


def _plan_band(prior_mean, prior_std):
    """f32 prior over every offset, exactly as the reference computes it,
    and the band of offsets whose scores can round exp() away from 1.0."""
    d = np.arange(-(N - 1), N, dtype=np.float32)
    ps = np.float32(prior_std)
    pm = np.float32(prior_mean)
    prior = (
        np.float32(1.0)
        / ps
        / np.sqrt(np.float32(2.0) * np.float32(PI))
        * np.exp(np.float32(-0.5) * (d - pm) ** 2 / ps**2)
    ).astype(np.float32)
    # |score| <= |prior| * |q.k*scale| ; bound the latter by 1024 (actual
    # max is ~7 for these glorot inputs).  exp(x) rounds to 1.0f for
    # |x| < 2^-26; use 2^-27 for margin.
    sig = np.abs(prior) * 1024.0 >= 2.0**-27
    if not sig.any():
        dlo, dhi = 0, 0
    else:
        dlo = int(d[sig].min())
        dhi = int(d[sig].max())
    return prior, dlo, dhi


def _window_geometry(dlo, dhi):
    """Per-quarter window starts ws4[64] plus deduplicated per-group
    prior patterns.  Pattern key for a group is (gt, rel offsets...) of
    its quarter-windows relative to the group's base row."""
    span = dhi - dlo
    win = QR + span + 1
    win = max(48, ((win + 15) // 16) * 16)
    assert win <= 192, f"prior band too wide for quarter-banded kernel: {dlo}..{dhi}"
    extra = win - (QR + span)
    ws4 = []
    for h in range(NQ):
        ws = min(max(h * QR + dlo - extra // 2, 0), N - win)
        lo_need = max(0, h * QR + dlo)
        hi_need = min(N - 1, h * QR + QR - 1 + dhi)
        assert ws <= lo_need and hi_need < ws + win, (h, ws, lo_need, hi_need)
        ws4.append(ws)
    gkeys = []
    for g, gt in enumerate(GROUPS):
        t0 = GSTART[g]
        base = t0 * P
        gkeys.append((gt,) + tuple(ws4[4 * t0 + i] - base for i in range(4 * gt)))
    key_vals = sorted(set(gkeys))
    key_idx = [key_vals.index(k) for k in gkeys]
    key_off = {}
    off = 0
    for k in key_vals:
        key_off[k] = off
        off += k[0] * win
    return win, ws4, key_vals, key_idx, key_off, off


def _build(win, ws4, key_idx, key_off_list, pat_cols, use_bias):
    nc = bacc.Bacc()
    GWMAX = max(GROUPS) * win

    # f32 consts: c1 | wsm | ii | bq8 | bk8
    O_C1, O_WS, O_II, O_BQ = 0, NT, 2 * NT, 3 * NT
    CW = 3 * NT + 2
    # bf16 consts: patterns then j0
    PJW = pat_cols + GWMAX
    O_J0 = pat_cols

    xt_d = nc.dram_tensor("xt", [P, DCH * N], F8, kind="ExternalInput")
    w2_d = nc.dram_tensor("w2", [P, 2 * DCH * MD], F8, kind="ExternalInput")
    cs_d = nc.dram_tensor("cst", [P, CW], F32, kind="ExternalInput")
    pj_d = nc.dram_tensor("pj", [P, PJW], BF16, kind="ExternalInput")
    y_d = nc.dram_tensor("y", [P, NT], F32, kind="ExternalOutput")

    with tile.TileContext(nc) as tc:
        with (
            tc.tile_pool(name="const", bufs=1) as const,
            tc.tile_pool(name="psum_proj", bufs=3, space="PSUM") as psum_proj,
            tc.tile_pool(name="psum_band", bufs=2, space="PSUM") as psum_band,
            tc.tile_pool(name="band_sp", bufs=2) as sp_pool,
            tc.tile_pool(name="band_ee", bufs=2) as ee_pool,
            tc.tile_pool(name="comb", bufs=1) as comb,
        ):
            # ---- engine warmups (emitted first so DVE/PE start at body
            # entry) ----
            # PE: junk matmuls flip the HAM clock gate (full speed needs
            # ~3us of continuous busy) while the input DMAs land.  ACT:
            # one tiny Exp pulls the 1.3us ACT_TABLE_LOAD off the
            # critical path.
            wtile = const.tile([P, GWMAX], BF16, tag="warm_w")
            nc.vector.memset(wtile, 0.0)
            wact_in = const.tile([P, 1], F32, tag="warm_a")
            nc.vector.memset(wact_in, 0.0)
            wact_out = const.tile([P, 1], F32, tag="warm_ao")
            nc.scalar.activation(
                out=wact_out, in_=wact_in, func=mybir.ActivationFunctionType.Exp
            )

            # ---- input DMAs: sync carries w2 + the c0 half of x, gpsimd
            # carries c1 + patterns + consts; ACT issues none so it is
            # free for evictions/exp.  Two big (2KB/partition) x
            # transfers beat four small ones: DMA here is
            # descriptor-latency-bound, not byte-bound. ----
            w2_s = const.tile([P, 2 * DCH * MD], F8, tag="w2")
            xt_s = const.tile([P, DCH * N], F8, tag="xt")
            pj_s = const.tile([P, PJW], BF16, tag="pj")
            cs_s = const.tile([P, CW], F32, tag="cst")
            nc.sync.dma_start(out=w2_s, in_=w2_d[:, :])
            nc.sync.dma_start(out=xt_s[:, 0:N], in_=xt_d[:, 0:N])  # c0
            nc.gpsimd.dma_start(out=xt_s[:, N : 2 * N], in_=xt_d[:, N : 2 * N])  # c1
            nc.gpsimd.dma_start(out=pj_s, in_=pj_d[:, :])
            nc.gpsimd.dma_start(out=cs_s, in_=cs_d[:, :])

            for _ in range(7):
                wps = psum_band.tile([P, GWMAX], F32, tag="band")
                nc.tensor.matmul(
                    wps, lhsT=wtile[:, :P], rhs=wtile, start=True, stop=True
                )

            qkT = const.tile([P, 2 * N], BF16, tag="qkT")  # q | k
            # interleaved sums: col 2t = sum_e[t], 2t+1 = sum_ec[t]
            sums = const.tile([P, 2 * NT], BF16, tag="sums")

            # ---- projection chunk: q and k into one [P, 1024] psum pair,
            # each a single fp8 DoubleRow matmul (contraction pairs are the
            # two D-halves), then ONE fused eviction ----
            EVICT_ENG = ["act", "dve", "act", "act"]

            def emit_proj(n4):
                ps_t = psum_proj.tile([P, 2 * PROJ_CHUNK], F32, tag="proj")
                rhs3 = xt_s[:].rearrange("p (c j) -> p c j", c=DCH)[
                    :, :, n4 * PROJ_CHUNK : (n4 + 1) * PROJ_CHUNK
                ]
                for pj in range(2):  # 0=q, 1=k
                    lhsT3 = w2_s[
                        :, 2 * pj * MD : (2 * pj + 2) * MD
                    ].rearrange("p (c m) -> p c m", c=DCH)
                    nc.tensor.matmul(
                        ps_t[:, pj * PROJ_CHUNK : (pj + 1) * PROJ_CHUNK],
                        lhsT=lhsT3,
                        rhs=rhs3,
                        start=True,
                        stop=True,
                        perf_mode=mybir.MatmulPerfMode.DoubleRow,
                    )
                # fused eviction: [P, 2, 512] view of qkT at (q, k) slices
                dst = qkT[:].rearrange("p (s j) -> p s j", s=2)[
                    :, :, n4 * PROJ_CHUNK : (n4 + 1) * PROJ_CHUNK
                ]
                src = ps_t[:].rearrange("p (s j) -> p s j", s=2)
                eng = EVICT_ENG[n4]
                if use_bias:
                    # per-partition bias differs for q and k: two ops
                    for pj in range(2):
                        b_s = cs_s[:, O_BQ + pj : O_BQ + pj + 1]
                        d1 = qkT[:, pj * N + n4 * PROJ_CHUNK : pj * N + (n4 + 1) * PROJ_CHUNK]
                        s1 = ps_t[:, pj * PROJ_CHUNK : (pj + 1) * PROJ_CHUNK]
                        if eng == "act":
                            nc.scalar.activation(
                                out=d1, in_=s1,
                                func=mybir.ActivationFunctionType.Identity,
                                bias=b_s, scale=1.0,
                            )
                        else:
                            nc.vector.tensor_scalar_add(d1, s1, b_s)
                else:
                    if eng == "act":
                        nc.scalar.copy(out=dst, in_=src)
                    else:
                        nc.vector.tensor_copy(dst, src)

            # ---- band group: 4*gt quarter matmuls, postproc in one pass ----
            def emit_group(g, defer_reduce=False):
                t0, gt = GSTART[g], GROUPS[g]
                gw = gt * win
                ps_full = psum_band.tile([P, GWMAX], F32, tag="band")
                ps_s = ps_full[:, :gw]
                for tb in range(gt):
                    t = t0 + tb
                    for qd in range(4):
                        ws = ws4[4 * t + qd]
                        nc.tensor.matmul(
                            ps_s[qd * QR : (qd + 1) * QR, tb * win : (tb + 1) * win],
                            lhsT=qkT[:, t * P + qd * QR : t * P + (qd + 1) * QR],
                            rhs=qkT[:, N + ws : N + ws + win],
                            start=True,
                            stop=True,
                            tile_position=(0, qd * QR),
                        )
                pat = pj_s[:, key_off_list[g] : key_off_list[g] + gw]
                sp_full = sp_pool.tile([P, GWMAX], BF16, tag="sp")
                sp_t = sp_full[:, :gw]
                nc.vector.tensor_mul(sp_t, ps_s, pat)
                ee_full = ee_pool.tile([P, 2 * GWMAX], BF16, tag="ee")
                ee_t = ee_full[:, : 2 * gw]
                nc.scalar.activation(
                    out=ee_t[:, :gw], in_=sp_t,
                    func=mybir.ActivationFunctionType.Exp,
                )
                nc.gpsimd.tensor_mul(
                    ee_t[:, gw : 2 * gw], ee_t[:, :gw], pj_s[:, O_J0 : O_J0 + gw]
                )
                # one reduce for both sums; out cols interleave as
                # (kind, tile) -> 2*(t0+tb)+kind via a [2, gt] out view
                out_ap = sums[:, 2 * t0 : 2 * (t0 + gt)].rearrange(
                    "p (t k) -> p k t", k=2
                )
                # bf16 sums: worst-case 0.4% of ~8e3 is ~0.02 abs on a
                # +-1023-scale output (tolerance 2e-2 rel) — safe.
                def do_reduce():
                    with nc.allow_low_precision("bf16 window sums, ~1e-5 rel out err"):
                        nc.vector.tensor_reduce(
                            out=out_ap,
                            in_=ee_t.rearrange("p (t w) -> p t w", w=win),
                            axis=mybir.AxisListType.X,
                            op=mybir.AluOpType.add,
                        )

                if defer_reduce:
                    return do_reduce
                do_reduce()

            # ---- combine: out = (c1 + sum_ec + ws*sum_e)/(N-win+sum_e) - i ----
            c1_s = cs_s[:, O_C1 : O_C1 + NT]
            ws_s = cs_s[:, O_WS : O_WS + NT]
            ii_s = cs_s[:, O_II : O_II + NT]
            outv2 = comb.tile([P, NT], F32, tag="outv2")

            def emit_combine(sl):
                # short serial spine on DVE (t0 -> rec -> outv -> outv2 with
                # no cross-engine hops); the numerator builds on Pool in
                # parallel
                w = sl.stop - sl.start
                se = sums[:, 2 * sl.start : 2 * sl.stop].rearrange(
                    "p (t k) -> p t k", k=2
                )[:, :, 0]
                sec = sums[:, 2 * sl.start : 2 * sl.stop].rearrange(
                    "p (t k) -> p t k", k=2
                )[:, :, 1]
                tmp = comb.tile([P, w], F32, tag="tmp")
                nc.gpsimd.tensor_mul(tmp, ws_s[:, sl], se)
                num = comb.tile([P, w], F32, tag="num")
                nc.gpsimd.tensor_add(num, c1_s[:, sl], sec)
                num2 = comb.tile([P, w], F32, tag="num2")
                nc.gpsimd.tensor_add(num2, num, tmp)
                t0 = comb.tile([P, w], F32, tag="t0")
                nc.vector.tensor_scalar_add(t0, se, float(N - win))
                rec = comb.tile([P, w], F32, tag="rec")
                nc.vector.reciprocal(rec, t0)
                outv = comb.tile([P, w], F32, tag="outv")
                nc.vector.tensor_mul(outv, num2, rec)
                nc.vector.tensor_sub(outv2[:, sl], outv, ii_s[:, sl])
                nc.sync.dma_start(out=y_d[:, sl], in_=outv2[:, sl])

            emit_proj(0)
            emit_group(0)  # tiles 0-2, k cols < 512
            emit_proj(1)
            emit_group(1)  # tiles 3-6, k < 1024
            emit_proj(2)
            emit_group(2)  # tiles 7-10, k < 1536
            emit_proj(3)
            emit_group(3)  # tiles 11-13
            red4 = emit_group(4, defer_reduce=True)  # tiles 14-15
            emit_combine(slice(0, 14))  # DVE spine runs under g4's exp/ej
            red4()
            emit_combine(slice(14, NT))  # short final tail

    nc.finalize()
    return nc


def kernel(x, Wq, bq, Wk, bk, prior_mean, prior_std):
    global last_run
    x = np.asarray(x, dtype=np.float32)
    Wq = np.asarray(Wq, dtype=np.float32)
    Wk = np.asarray(Wk, dtype=np.float32)
    bq = np.asarray(bq, dtype=np.float32)
    bk = np.asarray(bk, dtype=np.float32)

    prior, dlo, dhi = _plan_band(
        float(np.asarray(prior_mean)[0]), float(np.asarray(prior_std)[0])
    )
    win, ws4, key_vals, key_idx, key_off, pat_cols = _window_geometry(dlo, dhi)
    use_bias = bool(np.any(bq != 0.0) or np.any(bk != 0.0))
    key_off_list = [key_off[key_vals[key_idx[g]]] for g in range(len(GROUPS))]

    ckey = (win, tuple(ws4), tuple(key_idx), use_bias)
    if ckey not in _cache:
        _cache[ckey] = _build(win, ws4, key_idx, key_off_list, pat_cols, use_bias)
    nc = _cache[ckey]

    bf = ml_dtypes.bfloat16
    f8 = ml_dtypes.float8_e4m3
    scale = np.float32(MD**-0.5) / np.float32(WSCALE * WSCALE)
    GWMAX = max(GROUPS) * win

    # prior*scale patterns per distinct key, then j0
    p_idx = np.arange(P)[:, None]
    c_idx = np.arange(win)[None, :]
    quad = np.arange(P) // QR  # quarter index of each partition
    pj = np.zeros((P, pat_cols + GWMAX), np.float32)
    for kv in key_vals:
        gt, rel = kv[0], kv[1:]
        off = key_off[kv]
        for tb in range(gt):
            relcol = np.asarray(rel)[4 * tb + quad][:, None]
            dm = c_idx + relcol - P * tb - p_idx
            pj[:, off + tb * win : off + (tb + 1) * win] = np.where(
                (dm >= dlo) & (dm <= dhi), prior[dm + N - 1] * scale, np.float32(0.0)
            )
    pj[:, pat_cols:] = np.tile(np.arange(win, dtype=np.float32), max(GROUPS))[None, :]

    sumj_all = float(N * (N - 1) // 2)
    c1 = np.zeros((P, NT), np.float32)
    wsm = np.zeros((P, NT), np.float32)
    ii = np.zeros((P, NT), np.float32)
    ws4a = np.asarray(ws4, np.float32)
    for t in range(NT):
        wsv = ws4a[4 * t + quad]
        c1[:, t] = sumj_all - (win * wsv + win * (win - 1) // 2)
        wsm[:, t] = wsv
        ii[:, t] = t * P + np.arange(P)

    # f32 consts: c1 | wsm | ii | 8*bq | 8*bk  (weights ship pre-scaled x8,
    # so the bias folded into the eviction must match)
    cst = np.ascontiguousarray(
        np.concatenate(
            [
                c1,
                wsm,
                ii,
                np.float32(WSCALE) * bq.reshape(P, 1),
                np.float32(WSCALE) * bk.reshape(P, 1),
            ],
            axis=1,
        ).astype(np.float32)
    )
    pj16 = np.ascontiguousarray(pj.astype(bf))

    # weights: wq chunks then wk chunks, [P, 4*MD], fp8 at x8 scale
    wq_h = (Wq * WSCALE).reshape(DCH, P, MD).transpose(1, 0, 2).reshape(P, DCH * MD)
    wk_h = (Wk * WSCALE).reshape(DCH, P, MD).transpose(1, 0, 2).reshape(P, DCH * MD)
    w2_h = np.ascontiguousarray(np.concatenate([wq_h, wk_h], axis=1)).astype(f8)

    in_maps = []
    for core in range(NCORES):
        xb = x[core]  # [N, D]
        # xt[p, c*N + j] = x[j, c*128 + p]
        xt_h = np.ascontiguousarray(
            xb.T.reshape(DCH, P, N).transpose(1, 0, 2).reshape(P, DCH * N)
        ).astype(f8)
        in_maps.append({"xt": xt_h, "w2": w2_h, "cst": cst, "pj": pj16})

    res = run_bass_kernel_spmd(nc, in_maps, list(range(NCORES)))
    last_run = (nc, in_maps)
    # y[p, t] = out[128t + p]  ->  out = y.T.flatten()
    out = np.stack(
        [res.results[c]["y"].T.reshape(-1) for c in range(NCORES)], axis=0
    )
    return out.astype(np.float32)


# revision 24
# speedup vs baseline: 1.1764x; 1.0185x over previous
"""Trainium2 Bass kernel for nn_DistanceLayer (gaussian-prior distance attention).

Math: out[b,i] = sum_j softmax_j(q_i.k_j * MD^-0.5 * prior(j-i))[j] * (j-i)

The gaussian prior (std=1) underflows so fast in f32 that outside a small
band |j-i| <= 7 the f32 score is exactly 0, so exp(score) is exactly 1.0.
Each softmax row is a small band of "interesting" values plus a uniform
far field with closed-form sums:

    T0_i = (N - win) + sum_window exp(s)            (denominator)
    T1_i = C1_i + sum_window exp(s)*jrel + ws_i * sum_window exp(s)
    out_i = T1_i / T0_i - i

where C1_i = sum_all_j j - sum_window_i j (exact ints in f32) and ws_i is
the window start of row i's 32-row quarter.  In-window far entries have
score exactly 0 (prior pattern is 0 outside the band) and contribute
exp(0)=1, which the constants account for.

v4 layout/schedule:
  - 32-row QUARTER windows (win=48 vs 80 for 64-row halves): 40% fewer
    score elements, so the DVE-locked postprocessing (prior-mul and the
    window reduces, which have no 2x dtype mode) fits the engine budget.
    PE pays ~64 small band matmuls, but has DoubleRow headroom.
  - fp8e4m3 x/weights with DoubleRow matmuls: one PE instruction per
    (chunk, projection) at 0.5 cycles/row.
  - q and k of a chunk share one [P, 2, 512] PSUM pair and evict in a
    single fused op (ACT-heavy split).
  - band groups of (3,4,4,3,2) tiles: group 0 only needs k chunk 0 so
    postprocessing starts at the first eviction, and the last group's
    serial tail is short.  exp output and e*jrel share one tile; ONE
    reduce per group yields both sums interleaved (col 2t / 2t+1).
  - input DMAs ordered for earliest projection unlock: sync (xtA, xtC,
    patterns), scalar (w2, xtB, xtD), gpsimd (combine consts); PE junk
    matmuls ramp the 0.65->2.4GHz clock until real data lands.

Sharding: pure data-parallel over batch B=8 across the 8 cores.
"""

import sys

sys.path.insert(0, "/opt/trn_rl_repo")

import ml_dtypes
import numpy as np

import concourse.bacc as bacc
import concourse.tile as tile
from concourse import mybir
from concourse.bass_utils import run_bass_kernel_spmd

B, N, D, MD = 8, 2048, 256, 128
NCORES = 8
P = 128
QR = 32  # quarter height
NQ = N // QR  # 64 quarters
NT = N // P  # 16 row tiles
GROUPS = (3, 4, 4, 3, 2)  # band group sizes in tiles
GSTART = (0, 3, 7, 11, 14)
# group g's k windows reach into proj chunk UNLOCK[g]; emitted after it
UNLOCK = (0, 1, 2, 3, 3)
DCH = D // P  # 2 contraction chunks
PROJ_CHUNK = 512
NPC = N // PROJ_CHUNK  # 4 projection column chunks
PI = 3.1415926  # matches reference
WSCALE = 8.0  # fp8 weight pre-scale; pattern divides the x64 back out
F32 = mybir.dt.float32
BF16 = mybir.dt.bfloat16
F8 = mybir.dt.float8e4

_cache = {}
# exposed for test harness profiling: (nc, in_maps)
last_run = None


def _plan_band(prior_mean, prior_std):
    """f32 prior over every offset, exactly as the reference computes it,
    and the band of offsets whose scores can round exp() away from 1.0."""
    d = np.arange(-(N - 1), N, dtype=np.float32)
    ps = np.float32(prior_std)
    pm = np.float32(prior_mean)
    prior = (
        np.float32(1.0)
        / ps
        / np.sqrt(np.float32(2.0) * np.float32(PI))
        * np.exp(np.float32(-0.5) * (d - pm) ** 2 / ps**2)
    ).astype(np.float32)
    # |score| <= |prior| * |q.k*scale| ; bound the latter by 1024 (actual
    # max is ~7 for these glorot inputs).  exp(x) rounds to 1.0f for
    # |x| < 2^-26; use 2^-27 for margin.
    sig = np.abs(prior) * 1024.0 >= 2.0**-27
    if not sig.any():
        dlo, dhi = 0, 0
    else:
        dlo = int(d[sig].min())
        dhi = int(d[sig].max())
    return prior, dlo, dhi


def _window_geometry(dlo, dhi):
    """Per-quarter window starts ws4[64] plus deduplicated per-group
    prior patterns.  Pattern key for a group is (gt, rel offsets...) of
    its quarter-windows relative to the group's base row."""
    span = dhi - dlo
    win = QR + span + 1
    win = max(48, ((win + 15) // 16) * 16)
    assert win <= 192, f"prior band too wide for quarter-banded kernel: {dlo}..{dhi}"
    extra = win - (QR + span)
    ws4 = []
    for h in range(NQ):
        ws = min(max(h * QR + dlo - extra // 2, 0), N - win)
        lo_need = max(0, h * QR + dlo)
        hi_need = min(N - 1, h * QR + QR - 1 + dhi)
        assert ws <= lo_need and hi_need < ws + win, (h, ws, lo_need, hi_need)
        ws4.append(ws)
    gkeys = []
    for g, gt in enumerate(GROUPS):
        t0 = GSTART[g]
        base = t0 * P
        gkeys.append((gt,) + tuple(ws4[4 * t0 + i] - base for i in range(4 * gt)))
    key_vals = sorted(set(gkeys))
    key_idx = [key_vals.index(k) for k in gkeys]
    key_off = {}
    off = 0
    for k in key_vals:
        key_off[k] = off
        off += k[0] * win
    return win, ws4, key_vals, key_idx, key_off, off


def _build(win, ws4, key_idx, key_off_list, pat_cols, use_bias):
    nc = bacc.Bacc()
    GWMAX = max(GROUPS) * win

    # f32 consts: c1 | wsm | ii | bq8 | bk8
    O_C1, O_WS, O_II, O_BQ = 0, NT, 2 * NT, 3 * NT
    CW = 3 * NT + 2
    # bf16 consts: patterns then j0
    PJW = pat_cols + GWMAX
    O_J0 = pat_cols

    xt_d = nc.dram_tensor("xt", [P, DCH * N], F8, kind="ExternalInput")
    w2_d = nc.dram_tensor("w2", [P, 2 * DCH * MD], F8, kind="ExternalInput")
    cs_d = nc.dram_tensor("cst", [P, CW], F32, kind="ExternalInput")
    pj_d = nc.dram_tensor("pj", [P, PJW], BF16, kind="ExternalInput")
    y_d = nc.dram_tensor("y", [P, NT], F32, kind="ExternalOutput")

    with tile.TileContext(nc) as tc:
        with (
            tc.tile_pool(name="const", bufs=1) as const,
            tc.tile_pool(name="psum_proj", bufs=3, space="PSUM") as psum_proj,
            tc.tile_pool(name="psum_band", bufs=2, space="PSUM") as psum_band,
            tc.tile_pool(name="band_sp", bufs=2) as sp_pool,
            tc.tile_pool(name="band_ee", bufs=2) as ee_pool,
            tc.tile_pool(name="comb", bufs=1) as comb,
        ):
            # ---- engine warmups (emitted first so DVE/PE start at body
            # entry) ----
            # PE: junk matmuls flip the HAM clock gate (full speed needs
            # ~3us of continuous busy) while the input DMAs land.  ACT:
            # one tiny Exp pulls the 1.3us ACT_TABLE_LOAD off the
            # critical path.
            wtile = const.tile([P, GWMAX], BF16, tag="warm_w")
            nc.vector.memset(wtile, 0.0)
            wact_in = const.tile([P, 1], F32, tag="warm_a")
            nc.vector.memset(wact_in, 0.0)
            wact_out = const.tile([P, 1], F32, tag="warm_ao")
            nc.scalar.activation(
                out=wact_out, in_=wact_in, func=mybir.ActivationFunctionType.Exp
            )

            # ---- input DMAs: sync carries w2 + the c0 half of x, gpsimd
            # carries c1 + patterns + consts; ACT issues none so it is
            # free for evictions/exp.  Two big (2KB/partition) x
            # transfers beat four small ones: DMA here is
            # descriptor-latency-bound, not byte-bound. ----
            w2_s = const.tile([P, 2 * DCH * MD], F8, tag="w2")
            xt_s = const.tile([P, DCH * N], F8, tag="xt")
            pj_s = const.tile([P, PJW], BF16, tag="pj")
            cs_s = const.tile([P, CW], F32, tag="cst")
            half = N // 2
            nc.sync.dma_start(out=w2_s, in_=w2_d[:, :])
            nc.sync.dma_start(out=xt_s[:, 0:half], in_=xt_d[:, 0:half])  # c0 a
            nc.gpsimd.dma_start(  # c1 a
                out=xt_s[:, N : N + half], in_=xt_d[:, N : N + half]
            )
            nc.sync.dma_start(out=xt_s[:, half:N], in_=xt_d[:, half:N])  # c0 b
            nc.gpsimd.dma_start(  # c1 b
                out=xt_s[:, N + half : 2 * N], in_=xt_d[:, N + half : 2 * N]
            )
            nc.gpsimd.dma_start(out=pj_s, in_=pj_d[:, :])
            nc.gpsimd.dma_start(out=cs_s, in_=cs_d[:, :])

            for _ in range(10):
                wps = psum_band.tile([P, GWMAX], F32, tag="band")
                nc.tensor.matmul(
                    wps, lhsT=wtile[:, :P], rhs=wtile, start=True, stop=True
                )

            qkT = const.tile([P, 2 * N], BF16, tag="qkT")  # q | k
            # interleaved sums: col 2t = sum_e[t], 2t+1 = sum_ec[t]
            sums = const.tile([P, 2 * NT], BF16, tag="sums")

            # ---- projection chunk: q and k into one [P, 1024] psum pair,
            # each a single fp8 DoubleRow matmul (contraction pairs are the
            # two D-halves).  Early chunks evict split (ACT does k, DVE
            # does q, in parallel) to unlock the first band groups sooner;
            # later chunks evict fused on ACT. ----
            EVICT_ENG = ["split", "split", "act", "act"]

            def emit_proj(n4):
                ps_t = psum_proj.tile([P, 2 * PROJ_CHUNK], F32, tag="proj")
                rhs3 = xt_s[:].rearrange("p (c j) -> p c j", c=DCH)[
                    :, :, n4 * PROJ_CHUNK : (n4 + 1) * PROJ_CHUNK
                ]
                for pj in range(2):  # 0=q, 1=k
                    lhsT3 = w2_s[
                        :, 2 * pj * MD : (2 * pj + 2) * MD
                    ].rearrange("p (c m) -> p c m", c=DCH)
                    nc.tensor.matmul(
                        ps_t[:, pj * PROJ_CHUNK : (pj + 1) * PROJ_CHUNK],
                        lhsT=lhsT3,
                        rhs=rhs3,
                        start=True,
                        stop=True,
                        perf_mode=mybir.MatmulPerfMode.DoubleRow,
                    )
                # fused eviction: [P, 2, 512] view of qkT at (q, k) slices
                dst = qkT[:].rearrange("p (s j) -> p s j", s=2)[
                    :, :, n4 * PROJ_CHUNK : (n4 + 1) * PROJ_CHUNK
                ]
                src = ps_t[:].rearrange("p (s j) -> p s j", s=2)
                eng = EVICT_ENG[n4]
                if use_bias:
                    # per-partition bias differs for q and k: two ops
                    for pj in range(2):
                        b_s = cs_s[:, O_BQ + pj : O_BQ + pj + 1]
                        d1 = qkT[:, pj * N + n4 * PROJ_CHUNK : pj * N + (n4 + 1) * PROJ_CHUNK]
                        s1 = ps_t[:, pj * PROJ_CHUNK : (pj + 1) * PROJ_CHUNK]
                        if eng == "act":
                            nc.scalar.activation(
                                out=d1, in_=s1,
                                func=mybir.ActivationFunctionType.Identity,
                                bias=b_s, scale=1.0,
                            )
                        else:
                            nc.vector.tensor_scalar_add(d1, s1, b_s)
                else:
                    if eng == "act":
                        nc.scalar.copy(out=dst, in_=src)
                    elif eng == "dve":
                        nc.vector.tensor_copy(dst, src)
                    else:  # split: ACT takes k, DVE takes q, in parallel
                        nc.scalar.copy(
                            out=qkT[:, N + n4 * PROJ_CHUNK : N + (n4 + 1) * PROJ_CHUNK],
                            in_=ps_t[:, PROJ_CHUNK:],
                        )
                        nc.vector.tensor_copy(
                            qkT[:, n4 * PROJ_CHUNK : (n4 + 1) * PROJ_CHUNK],
                            ps_t[:, :PROJ_CHUNK],
                        )

            # ---- band group: 4*gt quarter matmuls, postproc in one pass ----
            def emit_group(g, defer_reduce=False):
                t0, gt = GSTART[g], GROUPS[g]
                gw = gt * win
                ps_full = psum_band.tile([P, GWMAX], F32, tag="band")
                ps_s = ps_full[:, :gw]
                for tb in range(gt):
                    t = t0 + tb
                    for qd in range(4):
                        ws = ws4[4 * t + qd]
                        nc.tensor.matmul(
                            ps_s[qd * QR : (qd + 1) * QR, tb * win : (tb + 1) * win],
                            lhsT=qkT[:, t * P + qd * QR : t * P + (qd + 1) * QR],
                            rhs=qkT[:, N + ws : N + ws + win],
                            start=True,
                            stop=True,
                            tile_position=(0, qd * QR),
                        )
                pat = pj_s[:, key_off_list[g] : key_off_list[g] + gw]
                sp_full = sp_pool.tile([P, GWMAX], BF16, tag="sp")
                sp_t = sp_full[:, :gw]
                nc.vector.tensor_mul(sp_t, ps_s, pat)
                ee_full = ee_pool.tile([P, 2 * GWMAX], BF16, tag="ee")
                ee_t = ee_full[:, : 2 * gw]
                nc.scalar.activation(
                    out=ee_t[:, :gw], in_=sp_t,
                    func=mybir.ActivationFunctionType.Exp,
                )
                nc.gpsimd.tensor_mul(
                    ee_t[:, gw : 2 * gw], ee_t[:, :gw], pj_s[:, O_J0 : O_J0 + gw]
                )
                # one reduce for both sums; out cols interleave as
                # (kind, tile) -> 2*(t0+tb)+kind via a [2, gt] out view
                out_ap = sums[:, 2 * t0 : 2 * (t0 + gt)].rearrange(
                    "p (t k) -> p k t", k=2
                )
                # bf16 sums: worst-case 0.4% of ~8e3 is ~0.02 abs on a
                # +-1023-scale output (tolerance 2e-2 rel) — safe.
                def do_reduce():
                    with nc.allow_low_precision("bf16 window sums, ~1e-5 rel out err"):
                        nc.vector.tensor_reduce(
                            out=out_ap,
                            in_=ee_t.rearrange("p (t w) -> p t w", w=win),
                            axis=mybir.AxisListType.X,
                            op=mybir.AluOpType.add,
                        )

                if defer_reduce:
                    return do_reduce
                do_reduce()

            # ---- combine: out = (c1 + sum_ec + ws*sum_e)/(N-win+sum_e) - i ----
            c1_s = cs_s[:, O_C1 : O_C1 + NT]
            ws_s = cs_s[:, O_WS : O_WS + NT]
            ii_s = cs_s[:, O_II : O_II + NT]
            outv2 = comb.tile([P, NT], F32, tag="outv2")

            def emit_combine(sl):
                # short serial spine on DVE (t0 -> rec -> outv -> outv2 with
                # no cross-engine hops); the numerator builds on Pool in
                # parallel
                w = sl.stop - sl.start
                se = sums[:, 2 * sl.start : 2 * sl.stop].rearrange(
                    "p (t k) -> p t k", k=2
                )[:, :, 0]
                sec = sums[:, 2 * sl.start : 2 * sl.stop].rearrange(
                    "p (t k) -> p t k", k=2
                )[:, :, 1]
                tmp = comb.tile([P, w], F32, tag="tmp")
                nc.gpsimd.tensor_mul(tmp, ws_s[:, sl], se)
                num = comb.tile([P, w], F32, tag="num")
                nc.gpsimd.tensor_add(num, c1_s[:, sl], sec)
                num2 = comb.tile([P, w], F32, tag="num2")
                nc.gpsimd.tensor_add(num2, num, tmp)
                t0 = comb.tile([P, w], F32, tag="t0")
                nc.vector.tensor_scalar_add(t0, se, float(N - win))
                rec = comb.tile([P, w], F32, tag="rec")
                nc.vector.reciprocal(rec, t0)
                outv = comb.tile([P, w], F32, tag="outv")
                nc.gpsimd.tensor_mul(outv, num2, rec)
                nc.gpsimd.tensor_sub(outv2[:, sl], outv, ii_s[:, sl])
                nc.sync.dma_start(out=y_d[:, sl], in_=outv2[:, sl])

            # shift-by-one: proj n+1's eviction is emitted before group
            # n-1's postprocessing so the engine FIFOs never make a band
            # group wait behind postproc of an earlier group
            emit_proj(0)
            emit_proj(1)
            emit_group(0)  # tiles 0-2, k cols < 512
            emit_group(1)  # tiles 3-6, k < 1024
            emit_proj(2)
            emit_group(2)  # tiles 7-10, k < 1536
            emit_proj(3)
            emit_group(3)  # tiles 11-13
            red4 = emit_group(4, defer_reduce=True)  # tiles 14-15
            emit_combine(slice(0, 14))  # DVE spine runs under g4's exp/ej
            red4()
            emit_combine(slice(14, NT))  # short final tail

    nc.finalize()
    return nc


def kernel(x, Wq, bq, Wk, bk, prior_mean, prior_std):
    global last_run
    x = np.asarray(x, dtype=np.float32)
    Wq = np.asarray(Wq, dtype=np.float32)
    Wk = np.asarray(Wk, dtype=np.float32)
    bq = np.asarray(bq, dtype=np.float32)
    bk = np.asarray(bk, dtype=np.float32)

    prior, dlo, dhi = _plan_band(
        float(np.asarray(prior_mean)[0]), float(np.asarray(prior_std)[0])
    )
    win, ws4, key_vals, key_idx, key_off, pat_cols = _window_geometry(dlo, dhi)
    use_bias = bool(np.any(bq != 0.0) or np.any(bk != 0.0))
    key_off_list = [key_off[key_vals[key_idx[g]]] for g in range(len(GROUPS))]

    ckey = (win, tuple(ws4), tuple(key_idx), use_bias)
    if ckey not in _cache:
        _cache[ckey] = _build(win, ws4, key_idx, key_off_list, pat_cols, use_bias)
    nc = _cache[ckey]

    bf = ml_dtypes.bfloat16
    f8 = ml_dtypes.float8_e4m3
    scale = np.float32(MD**-0.5) / np.float32(WSCALE * WSCALE)
    GWMAX = max(GROUPS) * win

    # prior*scale patterns per distinct key, then j0
    p_idx = np.arange(P)[:, None]
    c_idx = np.arange(win)[None, :]
    quad = np.arange(P) // QR  # quarter index of each partition
    pj = np.zeros((P, pat_cols + GWMAX), np.float32)
    for kv in key_vals:
        gt, rel = kv[0], kv[1:]
        off = key_off[kv]
        for tb in range(gt):
            relcol = np.asarray(rel)[4 * tb + quad][:, None]
            dm = c_idx + relcol - P * tb - p_idx
            pj[:, off + tb * win : off + (tb + 1) * win] = np.where(
                (dm >= dlo) & (dm <= dhi), prior[dm + N - 1] * scale, np.float32(0.0)
            )
    pj[:, pat_cols:] = np.tile(np.arange(win, dtype=np.float32), max(GROUPS))[None, :]

    sumj_all = float(N * (N - 1) // 2)
    c1 = np.zeros((P, NT), np.float32)
    wsm = np.zeros((P, NT), np.float32)
    ii = np.zeros((P, NT), np.float32)
    ws4a = np.asarray(ws4, np.float32)
    for t in range(NT):
        wsv = ws4a[4 * t + quad]
        c1[:, t] = sumj_all - (win * wsv + win * (win - 1) // 2)
        wsm[:, t] = wsv
        ii[:, t] = t * P + np.arange(P)

    # f32 consts: c1 | wsm | ii | 8*bq | 8*bk  (weights ship pre-scaled x8,
    # so the bias folded into the eviction must match)
    cst = np.ascontiguousarray(
        np.concatenate(
            [
                c1,
                wsm,
                ii,
                np.float32(WSCALE) * bq.reshape(P, 1),
                np.float32(WSCALE) * bk.reshape(P, 1),
            ],
            axis=1,
        ).astype(np.float32)
    )
    pj16 = np.ascontiguousarray(pj.astype(bf))

    # weights: wq chunks then wk chunks, [P, 4*MD], fp8 at x8 scale
    wq_h = (Wq * WSCALE).reshape(DCH, P, MD).transpose(1, 0, 2).reshape(P, DCH * MD)
    wk_h = (Wk * WSCALE).reshape(DCH, P, MD).transpose(1, 0, 2).reshape(P, DCH * MD)
    w2_h = np.ascontiguousarray(np.concatenate([wq_h, wk_h], axis=1)).astype(f8)

    in_maps = []
    for core in range(NCORES):
        xb = x[core]  # [N, D]
        # xt[p, c*N + j] = x[j, c*128 + p]
        xt_h = np.ascontiguousarray(
            xb.T.reshape(DCH, P, N).transpose(1, 0, 2).reshape(P, DCH * N)
        ).astype(f8)
        in_maps.append({"xt": xt_h, "w2": w2_h, "cst": cst, "pj": pj16})

    res = run_bass_kernel_spmd(nc, in_maps, list(range(NCORES)))
    last_run = (nc, in_maps)
    # y[p, t] = out[128t + p]  ->  out = y.T.flatten()
    out = np.stack(
        [res.results[c]["y"].T.reshape(-1) for c in range(NCORES)], axis=0
    )
    return out.astype(np.float32)
